# revision 1
# baseline (speedup 1.0000x reference)
"""BiLSTM-CRF forward loss on 8 Trainium2 cores (batch-parallel SPMD).

Layout/sharding summary (per core, b=8 examples of B=64):
- embedding gather -> x^T (PE transposes) -> L1 input-projection GEMM (bf16)
- L1 BiLSTM scan: col-tiled state-stationary matmuls (4 strips of 8
  partitions), gates layout [128p=(4 strips x 8b), 512f=(i|f|o|g)*128]
- L2 BiLSTM scan: both directions packed in one 128-partition tile
- linear -> logits^T [48, T*8] and logits [T*8, 48]
- CRF forward pass in exp-domain: alpha_t = (expT^T @ alpha) * exp(emit_t),
  one bf16 matmul + one DVE mul per step; logZ via log of the final sum
- gold path score via indirect-DMA gathers + selector matmuls
Outputs per core: [2, 8] fp32 (row0 joint, row1 logZ). Host sums
-(joint - logZ) over all 64 examples.
"""

import numpy as np

B, T, VOCAB, EMBED, HID, TAGS = 64, 512, 30000, 512, 1024, 48
H1, H2 = HID // 2, HID // 4  # 512, 256
BPC = B // 8  # batch per core = 8
NTOK = T * BPC  # 4096 tokens per core
LN48 = float(np.log(48.0))

_CACHE = {}


def _gate_perm(h):
    """Permutation p such that W[p] has strip layout:
    strip j (512 cols) = [i_j | f_j | o_j | g_j], each 128 units of gate
    blocks taken from pytorch (i,f,g,o) row order. h = per-dir hidden."""
    nj = (4 * h) // 512
    slots = [0, 1, 3, 2]  # i, f, o, g source gate index
    p = []
    for j in range(nj):
        for g_idx in slots:
            base = g_idx * h + j * 128
            p.extend(range(base, base + 128))
    return np.array(p, dtype=np.int64)


def _build_program():
    import concourse.bass as bass
    import concourse.tile as tile
    import concourse.mybir as mybir
    from concourse.vector_clock import ScopedClock, VectorClock
    from concourse.masks import make_identity

    def _patched_drain_and_barrier(self, tick_clock, wait_clock):
        # This container's walrus rejects >2 sem waits on one CTRL
        # instruction; split the kernel-tail drain waits into per-proc
        # NOP waits on the same (in-order) SP queue.
        vc = tick_clock.global_clock
        n = len(vc)
        for p in range(n):
            t = vc[p]
            if t > 0:
                vec = [0] * n
                vec[p] = t
                nop = self.nc.sync.nop()
                wait_clock.add_sem_waits(nop.ins, ScopedClock({None: VectorClock(vec)}))
        self.nc.sync.drain()
        self.nc.all_engine_barrier()
        popped = self.nc._tile_sem_poison_stack.pop()
        assert popped is self._sem_poison
        self.nc.clear_and_free_semaphores(list(self.sems.allocated().values()))
        self.nc.all_engine_barrier()

    tile.TileContext._drain_and_barrier = _patched_drain_and_barrier

    f32 = mybir.dt.float32
    bf16 = mybir.dt.bfloat16
    i32 = mybir.dt.int32
    ACT = mybir.ActivationFunctionType
    ADD = mybir.AluOpType.add
    MULT = mybir.AluOpType.mult

    nc = bass.Bass()
    PH = int(__import__("os").environ.get("KPHASES", "99"))

    # ---------------- inputs ----------------
    def din(name, shape, dt=f32):
        return nc.dram_tensor(name, shape, dt, kind="ExternalInput")

    embed_bf = din("embed_bf", [VOCAB, EMBED], bf16)
    seq_tok = din("seq_tok", [NTOK, 1], i32)
    idx_emit = din("idx_emit", [NTOK, 1], i32)
    idx_trans = din("idx_trans", [NTOK, 1], i32)
    idx_start = din("idx_start", [BPC, 1], i32)
    idx_end = din("idx_end", [BPC, 1], i32)
    wihT1 = din("wihT1", [2, EMBED, 4 * H1], bf16)
    whhT1 = din("whhT1", [2, H1, 4 * H1], bf16)
    bias1 = din("bias1", [2, 1, 4 * H1], bf16)
    wihT2 = din("wihT2", [2, HID, 4 * H2], bf16)
    whhT2 = din("whhT2", [2, H2, 4 * H2], bf16)
    bias2 = din("bias2", [2, 1, 4 * H2], bf16)
    linWT = din("linWT", [H2 * 2, TAGS], bf16)
    lin_b = din("lin_b", [1, TAGS], bf16)
    c0_l1 = din("c0_l1", [2, 128, 128])  # strip-packed
    h0T_l1 = din("h0T_l1", [2, H1, BPC], bf16)
    c0_l2 = din("c0_l2", [128, 128])  # both dirs strip-packed
    h0T_l2 = din("h0T_l2", [2, H2, BPC], bf16)
    trans_d = din("transitions", [TAGS, TAGS])
    start_d = din("start_trans", [TAGS, 1])
    end_d = din("end_trans", [TAGS, 1])
    sel_d = din("sel", [128, BPC])  # sel[p, b] = (p % 8 == b), fp32
    mask_d = din("mask_last", [128, 1])  # 1.0 except rows 120..127 -> 0.0

    out_d = nc.dram_tensor("out", [2, BPC], f32, kind="ExternalOutput")

    NM = NTOK // 128  # 32 token chunks

    with tile.TileContext(nc) as tc:
        with tc.tile_pool(name="dram", bufs=1, space="DRAM") as dpool, \
             tc.tile_pool(name="const", bufs=1) as cpool, \
             tc.tile_pool(name="persist", bufs=1) as ppool:

            xT_t = dpool.tile([EMBED, NTOK], bf16)          # 4 MB
            ih1_t = dpool.tile([2, T, 128, 512], bf16)      # 134 MB padded
            ih2_t = dpool.tile([T, 128, 512], bf16)         # 67 MB padded
            logits_t = dpool.tile([NTOK, TAGS], f32)

            ones_bf = cpool.tile([1, 512], bf16)
            nc.gpsimd.memset(ones_bf[:], 1.0)
            ones_f = cpool.tile([128, 1], f32)
            nc.gpsimd.memset(ones_f[:], 1.0)
            id8 = cpool.tile([128, 8], bf16)
            nc.gpsimd.memset(id8[:], 0.0)
            for j in range(4):
                make_identity(nc, id8[32 * j:32 * j + 8, :], nomemset=True)
            id128 = cpool.tile([128, 128], bf16)
            make_identity(nc, id128[:])
            sel_sb = cpool.tile([128, BPC], f32)
            nc.sync.dma_start(sel_sb[:], sel_d[:])
            mask_sb = cpool.tile([128, 1], f32)
            nc.sync.dma_start(mask_sb[:], mask_d[:])

            # h1T / h2T live in SBUF: [128, NTOK] bf16 tiles per 128-unit chunk
            h1T = [ppool.tile([128, NTOK], bf16, tag=f"h1T{i}", name=f"h1T{i}") for i in range(8)]
            h2T = [ppool.tile([128, NTOK], bf16, tag=f"h2T{i}", name=f"h2T{i}") for i in range(4)]

            # ================= P1: embedding gather + transpose =============
            with tc.tile_pool(name="p1", bufs=3) as sp, \
                 tc.tile_pool(name="p1p", bufs=4, space="PSUM") as psp:
                for m in range(NM if PH >= 1 else 0):
                    idx = sp.tile([128, 1], i32, tag="idx")
                    nc.sync.dma_start(idx[:], seq_tok[128 * m:128 * (m + 1), :])
                    xg = sp.tile([128, EMBED], bf16, tag="xg")
                    nc.gpsimd.indirect_dma_start(
                        out=xg[:], out_offset=None, in_=embed_bf[:],
                        in_offset=bass.IndirectOffsetOnAxis(ap=idx[:, :1], axis=0))
                    for e in range(EMBED // 128):
                        pt = psp.tile([128, 128], bf16, space="PSUM", tag="pt")
                        nc.tensor.transpose(out=pt[:], in_=xg[:, 128 * e:128 * (e + 1)],
                                            identity=id128[:])
                        xs = sp.tile([128, 128], bf16, tag="xs")
                        nc.vector.tensor_copy(xs[:], pt[:])
                        nc.sync.dma_start(
                            xT_t[128 * e:128 * (e + 1), 128 * m:128 * (m + 1)], xs[:])

            # ================= P2: L1 input GEMM ===========================
            with tc.tile_pool(name="w2", bufs=1) as wp, \
                 tc.tile_pool(name="p2", bufs=3) as sp, \
                 tc.tile_pool(name="p2p", bufs=4, space="PSUM") as psp:
                for d in range(2 if PH >= 2 else 0):
                    wt = [wp.tile([128, 4 * H1], bf16, tag=f"w1_{d}_{k}", name=f"w1_{d}_{k}") for k in range(4)]
                    for k in range(4):
                        nc.sync.dma_start(wt[k][:], wihT1[d, 128 * k:128 * (k + 1), :])
                    bt = wp.tile([1, 4 * H1], bf16, tag=f"b1_{d}")
                    nc.sync.dma_start(bt[:], bias1[d])
                    for m in range(NM):
                        xs = sp.tile([128, EMBED], bf16, tag="xs")
                        nc.sync.dma_start(
                            xs[:].rearrange("p (k n) -> p k n", k=4),
                            xT_t[:, 128 * m:128 * (m + 1)].rearrange(
                                "(k p) n -> p k n", p=128))
                        gsb = sp.tile([128, 4 * H1], bf16, tag="gsb")
                        for jj in range(4):
                            pg = psp.tile([128, 512], f32, space="PSUM", tag="pg")
                            for k in range(4):
                                nc.tensor.matmul(
                                    pg[:], lhsT=xs[:, 128 * k:128 * (k + 1)],
                                    rhs=wt[k][:, 512 * jj:512 * (jj + 1)],
                                    start=(k == 0), stop=False)
                            nc.tensor.matmul(
                                pg[:], lhsT=ones_bf[0:1, 0:128],
                                rhs=bt[0:1, 512 * jj:512 * (jj + 1)],
                                start=False, stop=True)
                            nc.scalar.copy(gsb[:, 512 * jj:512 * (jj + 1)], pg[:])
                        t0 = m * 16
                        for j in range(4):
                            dst = ih1_t[d, t0:t0 + 16].rearrange(
                                "t (j q) u -> t j q u", j=4)[:, j, 0:BPC, :]
                            nc.sync.dma_start(
                                dst, gsb[:, 512 * j:512 * (j + 1)])

            # ================= P3: L1 scans (fwd + bwd) ====================
            with tc.tile_pool(name="w3", bufs=1) as wp, \
                 tc.tile_pool(name="st3", bufs=1) as stp, \
                 tc.tile_pool(name="p3", bufs=4) as sp, \
                 tc.tile_pool(name="p3g", bufs=2, space="PSUM") as psg, \
                 tc.tile_pool(name="p3t", bufs=4, space="PSUM") as pst:
                whh = {}
                for d in range(2):
                    for k in range(4):
                        w = wp.tile([128, 4 * H1], bf16, tag=f"whh1_{d}_{k}")
                        nc.sync.dma_start(w[:], whhT1[d, 128 * k:128 * (k + 1), :])
                        whh[(d, k)] = w
                cS = {}
                hT0 = {}
                for d in range(2):
                    for par in range(2):
                        c = stp.tile([128, 128], f32, tag=f"c1_{d}_{par}")
                        cS[(d, par)] = c
                    nc.sync.dma_start(cS[(d, 0)][:], c0_l1[d])
                    h0 = stp.tile([128, 32], bf16, tag=f"h0T1_{d}")
                    for k in range(4):
                        nc.sync.dma_start(h0[:, 8 * k:8 * (k + 1)],
                                          h0T_l1[d, 128 * k:128 * (k + 1), :])
                    hT0[d] = h0

                for s in range(T if PH >= 3 else 0):
                    for d in range(2):
                        t = s if d == 0 else T - 1 - s
                        # lhsT source: previous hidden state transposed
                        if s == 0:
                            lhsT_of = lambda k, d=d: hT0[d][:, 8 * k:8 * (k + 1)]
                        else:
                            tp = (s - 1) if d == 0 else (T - s)
                            lhsT_of = (lambda k, d=d, tp=tp:
                                       h1T[d * 4 + k][:, 8 * tp:8 * tp + 8])
                        ih = sp.tile([128, 512], bf16, tag=f"ih1_{d}")
                        nc.sync.dma_start(ih[:], ih1_t[d, t])
                        pg = psg.tile([128, 512], f32, space="PSUM", tag=f"pg_{d}")
                        for jj in range(4):
                            for k in range(4):
                                nc.tensor.matmul(
                                    pg[32 * jj:32 * jj + 8, :], lhsT=lhsT_of(k),
                                    rhs=whh[(d, k)][:, 512 * jj:512 * (jj + 1)],
                                    start=(k == 0), stop=(k == 3),
                                    tile_position=(0, 32 * jj))
                        gsb = sp.tile([128, 512], bf16, tag=f"g1_{d}")
                        nc.vector.tensor_tensor(out=gsb[:], in0=pg[:], in1=ih[:], op=ADD)
                        sig = sp.tile([128, 384], bf16, tag=f"sig1_{d}")
                        nc.scalar.activation(sig[:], gsb[:, 0:384], ACT.Sigmoid)
                        tg = sp.tile([128, 128], bf16, tag=f"tg1_{d}")
                        nc.scalar.activation(tg[:], gsb[:, 384:512], ACT.Tanh)
                        c_old = cS[(d, s % 2)]
                        c_new = cS[(d, (s + 1) % 2)]
                        t1 = sp.tile([128, 128], f32, tag=f"t1_{d}")
                        nc.vector.tensor_tensor(out=t1[:], in0=sig[:, 128:256], in1=c_old[:], op=MULT)
                        t2 = sp.tile([128, 128], f32, tag=f"t2_{d}")
                        nc.vector.tensor_tensor(out=t2[:], in0=sig[:, 0:128], in1=tg[:], op=MULT)
                        nc.vector.tensor_tensor(out=c_new[:], in0=t1[:], in1=t2[:], op=ADD)
                        th = sp.tile([128, 128], bf16, tag=f"th1_{d}")
                        nc.scalar.activation(th[:], c_new[:], ACT.Tanh)
                        h = sp.tile([128, 128], bf16, tag=f"h1_{d}")
                        nc.vector.tensor_tensor(out=h[:], in0=sig[:, 256:384], in1=th[:], op=MULT)
                        for k in range(4):
                            pt = pst.tile([128, 8], bf16, space="PSUM", tag="pt3")
                            nc.tensor.transpose(out=pt[:], in_=h[32 * k:32 * k + 8, :],
                                                identity=id8[32 * k:32 * k + 8, :],
                                                tile_position=(32 * k, 0))
                            if k % 2 == 0:
                                nc.vector.tensor_copy(h1T[d * 4 + k][:, 8 * t:8 * t + 8], pt[:])
                            else:
                                nc.scalar.copy(h1T[d * 4 + k][:, 8 * t:8 * t + 8], pt[:])

            # ================= P4: L2 input GEMM ===========================
            with tc.tile_pool(name="w4", bufs=1) as wp, \
                 tc.tile_pool(name="p4", bufs=3) as sp, \
                 tc.tile_pool(name="p4p", bufs=4, space="PSUM") as psp:
                for d in range(2 if PH >= 4 else 0):
                    wt = [wp.tile([128, 4 * H2], bf16, tag=f"w2_{d}_{k}", name=f"w2_{d}_{k}") for k in range(8)]
                    for k in range(8):
                        nc.sync.dma_start(wt[k][:], wihT2[d, 128 * k:128 * (k + 1), :])
                    bt = wp.tile([1, 4 * H2], bf16, tag=f"b2_{d}")
                    nc.sync.dma_start(bt[:], bias2[d])
                    for m in range(NM):
                        gsb = sp.tile([128, 4 * H2], bf16, tag="g2sb")
                        for jj in range(2):
                            pg = psp.tile([128, 512], f32, space="PSUM", tag="pg4")
                            for k in range(8):
                                nc.tensor.matmul(
                                    pg[:], lhsT=h1T[k][:, 128 * m:128 * (m + 1)],
                                    rhs=wt[k][:, 512 * jj:512 * (jj + 1)],
                                    start=(k == 0), stop=False)
                            nc.tensor.matmul(
                                pg[:], lhsT=ones_bf[0:1, 0:128],
                                rhs=bt[0:1, 512 * jj:512 * (jj + 1)],
                                start=False, stop=True)
                            nc.scalar.copy(gsb[:, 512 * jj:512 * (jj + 1)], pg[:])
                        t0 = m * 16
                        for j in range(2):
                            dst = ih2_t[t0:t0 + 16].rearrange(
                                "t (j q) u -> t j q u", j=4)[:, 2 * d + j, 0:BPC, :]
                            nc.sync.dma_start(dst, gsb[:, 512 * j:512 * (j + 1)])

            # ================= P5: L2 scans (both dirs packed) =============
            with tc.tile_pool(name="w5", bufs=1) as wp, \
                 tc.tile_pool(name="st5", bufs=1) as stp, \
                 tc.tile_pool(name="p5", bufs=4) as sp, \
                 tc.tile_pool(name="p5g", bufs=2, space="PSUM") as psg, \
                 tc.tile_pool(name="p5t", bufs=4, space="PSUM") as pst:
                whh2 = {}
                for d in range(2):
                    for k in range(2):
                        w = wp.tile([128, 4 * H2], bf16, tag=f"whh2_{d}_{k}")
                        nc.sync.dma_start(w[:], whhT2[d, 128 * k:128 * (k + 1), :])
                        whh2[(d, k)] = w
                c2 = [stp.tile([128, 128], f32, tag=f"c2_{p}", name=f"c2_{p}") for p in range(2)]
                nc.sync.dma_start(c2[0][:], c0_l2[:])
                h0_2 = stp.tile([128, 32], bf16, tag="h0T2")
                for d in range(2):
                    for k in range(2):
                        nc.sync.dma_start(h0_2[:, 8 * (2 * d + k):8 * (2 * d + k) + 8],
                                          h0T_l2[d, 128 * k:128 * (k + 1), :])

                for s in range(T if PH >= 5 else 0):
                    ih = sp.tile([128, 512], bf16, tag="ih2")
                    tf, tb = s, T - 1 - s
                    nc.sync.dma_start(ih[0:64, :], ih2_t[tf, 0:64, :])
                    nc.sync.dma_start(ih[64:128, :], ih2_t[tb, 64:128, :])
                    pg = psg.tile([128, 512], f32, space="PSUM", tag="pg5")
                    for d in range(2):
                        t = tf if d == 0 else tb
                        for j in range(2):
                            for k in range(2):
                                if s == 0:
                                    lh = h0_2[:, 8 * (2 * d + k):8 * (2 * d + k) + 8]
                                else:
                                    tp = (s - 1) if d == 0 else (T - s)
                                    lh = h2T[2 * d + k][:, 8 * tp:8 * tp + 8]
                                nc.tensor.matmul(
                                    pg[32 * (2 * d + j):32 * (2 * d + j) + 8, :],
                                    lhsT=lh,
                                    rhs=whh2[(d, k)][:, 512 * j:512 * (j + 1)],
                                    start=(k == 0), stop=(k == 1),
                                    tile_position=(0, 32 * (2 * d + j)))
                    gsb = sp.tile([128, 512], bf16, tag="g5")
                    nc.vector.tensor_tensor(out=gsb[:], in0=pg[:], in1=ih[:], op=ADD)
                    sig = sp.tile([128, 384], bf16, tag="sig5")
                    nc.scalar.activation(sig[:], gsb[:, 0:384], ACT.Sigmoid)
                    tg = sp.tile([128, 128], bf16, tag="tg5")
                    nc.scalar.activation(tg[:], gsb[:, 384:512], ACT.Tanh)
                    c_old, c_new = c2[s % 2], c2[(s + 1) % 2]
                    t1 = sp.tile([128, 128], f32, tag="t15")
                    nc.vector.tensor_tensor(out=t1[:], in0=sig[:, 128:256], in1=c_old[:], op=MULT)
                    t2 = sp.tile([128, 128], f32, tag="t25")
                    nc.vector.tensor_tensor(out=t2[:], in0=sig[:, 0:128], in1=tg[:], op=MULT)
                    nc.vector.tensor_tensor(out=c_new[:], in0=t1[:], in1=t2[:], op=ADD)
                    th = sp.tile([128, 128], bf16, tag="th5")
                    nc.scalar.activation(th[:], c_new[:], ACT.Tanh)
                    h = sp.tile([128, 128], bf16, tag="h5")
                    nc.vector.tensor_tensor(out=h[:], in0=sig[:, 256:384], in1=th[:], op=MULT)
                    for q in range(4):  # q = 2*d + k
                        d, k = q // 2, q % 2
                        t = tf if d == 0 else tb
                        pt = pst.tile([128, 8], bf16, space="PSUM", tag="pt5")
                        nc.tensor.transpose(out=pt[:], in_=h[32 * q:32 * q + 8, :],
                                            identity=id8[32 * q:32 * q + 8, :],
                                            tile_position=(32 * q, 0))
                        if q % 2 == 0:
                            nc.vector.tensor_copy(h2T[q][:, 8 * t:8 * t + 8], pt[:])
                        else:
                            nc.scalar.copy(h2T[q][:, 8 * t:8 * t + 8], pt[:])

            # ================= P6: linear -> logitsT + logits ==============
            logitsT = ppool.tile([TAGS, NTOK], f32, tag="logitsT")
            Esb = ppool.tile([TAGS, NTOK], bf16, tag="Esb")
            with tc.tile_pool(name="w6", bufs=1) as wp, \
                 tc.tile_pool(name="p6", bufs=3) as sp, \
                 tc.tile_pool(name="p6p", bufs=4, space="PSUM") as psp:
                lw = [wp.tile([128, TAGS], bf16, tag=f"lw{k}", name=f"lw{k}") for k in range(4)]
                for k in range(4):
                    nc.sync.dma_start(lw[k][:], linWT[128 * k:128 * (k + 1), :])
                lb = wp.tile([1, TAGS], bf16, tag="lb")
                nc.sync.dma_start(lb[:], lin_b[:])
                # logitsT [48, NTOK]
                for n in range(NTOK // 512 if PH >= 6 else 0):
                    pg = psp.tile([TAGS, 512], f32, space="PSUM", tag="pl")
                    for k in range(4):
                        nc.tensor.matmul(pg[:], lhsT=lw[k][:],
                                         rhs=h2T[k][:, 512 * n:512 * (n + 1)],
                                         start=(k == 0), stop=False)
                    nc.tensor.matmul(pg[:], lhsT=lb[0:1, :], rhs=ones_bf[0:1, :],
                                     start=False, stop=True)
                    nc.scalar.copy(logitsT[:, 512 * n:512 * (n + 1)], pg[:])
                    nc.scalar.activation(Esb[:, 512 * n:512 * (n + 1)],
                                         pg[:], ACT.Exp)
                # logits [NTOK, 48] to DRAM for gathers
                for m in range(NM if PH >= 6 else 0):
                    pg = psp.tile([128, TAGS], f32, space="PSUM", tag="pl2")
                    for k in range(4):
                        nc.tensor.matmul(pg[:], lhsT=h2T[k][:, 128 * m:128 * (m + 1)],
                                         rhs=lw[k][:], start=(k == 0), stop=False)
                    nc.tensor.matmul(pg[:], lhsT=ones_bf[0:1, 0:128], rhs=lb[0:1, :],
                                     start=False, stop=True)
                    ls = sp.tile([128, TAGS], f32, tag="ls")
                    nc.scalar.copy(ls[:], pg[:])
                    nc.sync.dma_start(logits_t[128 * m:128 * (m + 1), :], ls[:])

            # ================= P7: CRF partition (exp domain) ==============
            with tc.tile_pool(name="p7", bufs=1) as sp, \
                 tc.tile_pool(name="p7a", bufs=4) as ap, \
                 tc.tile_pool(name="p7p", bufs=2, space="PSUM") as psp, \
                 tc.tile_pool(name="p7f", bufs=1, space="PSUM") as psf:
                tr = sp.tile([TAGS, TAGS], f32, tag="tr")
                nc.sync.dma_start(tr[:], trans_d[:])
                ETp = sp.tile([TAGS, TAGS], bf16, tag="ETp")
                nln48 = sp.tile([TAGS, 1], f32, tag="nln48")
                nc.gpsimd.memset(nln48[:], -LN48)
                nc.scalar.activation(ETp[:], tr[:], ACT.Exp, bias=nln48[:, 0:1])
                stv = sp.tile([TAGS, 1], f32, tag="stv")
                nc.sync.dma_start(stv[:], start_d[:])
                env = sp.tile([TAGS, 1], f32, tag="env")
                nc.sync.dma_start(env[:], end_d[:])
                eend = sp.tile([TAGS, 1], bf16, tag="eend")
                nc.scalar.activation(eend[:], env[:], ACT.Exp)

                alpha = ap.tile([TAGS, BPC], bf16, tag="alpha")
                nc.scalar.activation(alpha[:], logitsT[:, 0:BPC], ACT.Exp,
                                     bias=stv[:, 0:1])
                for t in range(1, T if PH >= 7 else 1):
                    pm = psp.tile([TAGS, BPC], f32, space="PSUM", tag="pm")
                    nc.tensor.matmul(pm[:], lhsT=ETp[:], rhs=alpha[:],
                                     start=True, stop=True)
                    alpha = ap.tile([TAGS, BPC], bf16, tag="alpha")
                    nc.vector.tensor_tensor(out=alpha[:], in0=pm[:],
                                            in1=Esb[:, BPC * t:BPC * (t + 1)], op=MULT)
                pf = psf.tile([1, BPC], f32, space="PSUM", tag="pf")
                nc.tensor.matmul(pf[:], lhsT=eend[:], rhs=alpha[:],
                                 start=True, stop=True)
                logz = sp.tile([1, BPC], f32, tag="logz")
                nc.scalar.activation(logz[:], pf[:], ACT.Ln)
                nc.vector.tensor_scalar_add(logz[:], logz[:], float((T - 1) * LN48))
                nc.sync.dma_start(out_d[1:2, :], logz[:])

            # ================= P8: gold path score =========================
            with tc.tile_pool(name="p8", bufs=3) as sp, \
                 tc.tile_pool(name="p8a", bufs=1) as aw, \
                 tc.tile_pool(name="p8p", bufs=2, space="PSUM") as psp:
                accW = aw.tile([128, NM], f32, tag="accW")
                lfl = logits_t[:].rearrange("n k -> (n k)").unsqueeze(1)
                tfl = trans_d[:].rearrange("a b -> (a b)").unsqueeze(1)
                for m in range(NM if PH >= 8 else 0):
                    ie = sp.tile([128, 1], i32, tag="ie")
                    nc.sync.dma_start(ie[:], idx_emit[128 * m:128 * (m + 1), :])
                    it = sp.tile([128, 1], i32, tag="it")
                    nc.sync.dma_start(it[:], idx_trans[128 * m:128 * (m + 1), :])
                    ge = sp.tile([128, 1], f32, tag="ge")
                    nc.gpsimd.indirect_dma_start(
                        out=ge[:], out_offset=None, in_=lfl,
                        in_offset=bass.IndirectOffsetOnAxis(ap=ie[:, :1], axis=0))
                    gt = sp.tile([128, 1], f32, tag="gt")
                    nc.gpsimd.indirect_dma_start(
                        out=gt[:], out_offset=None, in_=tfl,
                        in_offset=bass.IndirectOffsetOnAxis(ap=it[:, :1], axis=0))
                    if m == NM - 1:
                        nc.vector.tensor_tensor(out=gt[:], in0=gt[:],
                                                in1=mask_sb[:], op=MULT)
                    nc.vector.tensor_tensor(out=accW[:, m:m + 1], in0=ge[:],
                                            in1=gt[:], op=ADD)
                se16 = sp.tile([16, 1], f32, tag="se16")
                ist = sp.tile([BPC, 1], i32, tag="ist")
                nc.sync.dma_start(ist[:], idx_start[:])
                nc.gpsimd.indirect_dma_start(
                    out=se16[0:8, :], out_offset=None,
                    in_=start_d[:].rearrange("a b -> (a b)").unsqueeze(1),
                    in_offset=bass.IndirectOffsetOnAxis(ap=ist[:, :1], axis=0))
                ien = sp.tile([BPC, 1], i32, tag="ien")
                nc.sync.dma_start(ien[:], idx_end[:])
                nc.gpsimd.indirect_dma_start(
                    out=se16[8:16, :], out_offset=None,
                    in_=end_d[:].rearrange("a b -> (a b)").unsqueeze(1),
                    in_offset=bass.IndirectOffsetOnAxis(ap=ien[:, :1], axis=0))
                s1 = psp.tile([NM, BPC], f32, space="PSUM", tag="s1")
                nc.tensor.matmul(s1[:], lhsT=accW[:], rhs=sel_sb[:],
                                 start=True, stop=True)
                s1s = sp.tile([NM, BPC], f32, tag="s1s")
                nc.scalar.copy(s1s[:], s1[:])
                s2 = psp.tile([1, BPC], f32, space="PSUM", tag="s2")
                nc.tensor.matmul(s2[:], lhsT=ones_f[0:NM, 0:1], rhs=s1s[:],
                                 start=True, stop=False)
                nc.tensor.matmul(s2[:], lhsT=se16[:], rhs=sel_sb[0:16, :],
                                 start=False, stop=True)
                joint = sp.tile([1, BPC], f32, tag="joint")
                nc.scalar.copy(joint[:], s2[:])
                nc.sync.dma_start(out_d[0:1, :], joint[:])

    _split_waits(nc, maxw=int(__import__("os").environ.get("KMAXW", "1")))
    return nc


def _split_waits(nc, maxw=2):
    """This container's walrus rejects instructions carrying more than a
    couple of semaphore waits. Hoist extras onto preceding same-engine
    NoOps (engines execute their stream in order, so this preserves the
    happens-before)."""
    import concourse.mybir as mybir
    import bass_rust
    compute_ops = {"Matmult", "Activation", "TensorTensor", "TensorScalar",
                   "TensorCopy", "TensorReduce", "Memset", "Iota",
                   "AffineSelect", "TensorTensorScan", "Select"}
    n_added = 0
    for fn in nc.m.functions:
        for blk in fn.blocks:
            insts = list(blk.instructions)
            out = []
            dirty = False
            for inst in insts:
                mw = 2 if (maxw == 0 and str(inst.opcode) in compute_ops) else max(1, maxw)
                si = inst.sync_info
                if si is not None and len(si.on_wait) > mw:
                    waits = list(si.on_wait)
                    extra, keep = waits[:-mw], waits[-mw:]
                    for i in range(0, len(extra), mw):
                        nop = mybir.InstNoOp(
                            name=f"{inst.name}_hw{i}", ins=[], outs=[])
                        nop.engine = inst.engine
                        nop.sync_info = bass_rust.SyncInfo(
                            on_wait=extra[i:i + mw], on_update=[])
                        out.append(nop)
                        n_added += 1
                    inst.sync_info = bass_rust.SyncInfo(
                        on_wait=keep, on_update=list(si.on_update))
                    dirty = True
                out.append(inst)
            if dirty:
                blk.instructions = out
    return n_added


def _prep_inputs(inputs):
    import ml_dtypes
    bf = ml_dtypes.bfloat16

    g = {k: np.asarray(v) for k, v in inputs.items()}
    seq = g["sequences"].astype(np.int64)
    tags = g["tags"].astype(np.int64)

    p1 = _gate_perm(H1)
    p2 = _gate_perm(H2)

    shared = {}
    shared["embed_bf"] = np.ascontiguousarray(g["embed_table"].astype(bf))
    for d, sfx in enumerate(["1f", "1b"]):
        wih = g["w_ih" + sfx][p1]  # [2048, 512] permuted rows
        whh = g["w_hh" + sfx][p1]
        b = (g["b_ih" + sfx] + g["b_hh" + sfx])[p1]
        shared.setdefault("wihT1", np.zeros((2, EMBED, 4 * H1), bf))[d] = wih.T.astype(bf)
        shared.setdefault("whhT1", np.zeros((2, H1, 4 * H1), bf))[d] = whh.T.astype(bf)
        shared.setdefault("bias1", np.zeros((2, 1, 4 * H1), bf))[d] = b.astype(bf)[None]
    for d, sfx in enumerate(["2f", "2b"]):
        wih = g["w_ih" + sfx][p2]
        whh = g["w_hh" + sfx][p2]
        b = (g["b_ih" + sfx] + g["b_hh" + sfx])[p2]
        shared.setdefault("wihT2", np.zeros((2, HID, 4 * H2), bf))[d] = wih.T.astype(bf)
        shared.setdefault("whhT2", np.zeros((2, H2, 4 * H2), bf))[d] = whh.T.astype(bf)
        shared.setdefault("bias2", np.zeros((2, 1, 4 * H2), bf))[d] = b.astype(bf)[None]
    shared["linWT"] = np.ascontiguousarray(g["lin_w"].T.astype(bf))
    shared["lin_b"] = g["lin_b"].astype(bf)[None, :]
    shared["transitions"] = g["transitions"].astype(np.float32)
    shared["start_trans"] = g["start_trans"].astype(np.float32)[:, None]
    shared["end_trans"] = g["end_trans"].astype(np.float32)[:, None]
    sel = (np.arange(128)[:, None] % 8 == np.arange(8)[None, :]).astype(np.float32)
    shared["sel"] = sel
    mask = np.ones((128, 1), np.float32)
    mask[120:128] = 0.0
    shared["mask_last"] = mask

    in_maps = []
    for c in range(8):
        b0 = c * BPC
        m = dict(shared)
        sl = slice(b0, b0 + BPC)
        seq_c = seq[sl]  # [8, T]
        tags_c = tags[sl]
        tok_seq = seq_c.T.reshape(NTOK).astype(np.int32)  # token-major (t, b)
        tok_tags = tags_c.T.reshape(NTOK).astype(np.int64)
        m["seq_tok"] = tok_seq[:, None]
        m["idx_emit"] = (np.arange(NTOK, dtype=np.int64) * TAGS + tok_tags).astype(np.int32)[:, None]
        nxt = np.concatenate([tok_tags[BPC:], np.zeros(BPC, np.int64)])
        it = tok_tags * TAGS + nxt
        it[-BPC:] = 0
        m["idx_trans"] = it.astype(np.int32)[:, None]
        m["idx_start"] = tags_c[:, 0].astype(np.int32)[:, None]
        m["idx_end"] = tags_c[:, T - 1].astype(np.int32)[:, None]

        c0l1 = np.zeros((2, 128, 128), np.float32)
        h0l1 = np.zeros((2, H1, BPC), np.float32)
        for d in range(2):
            cc = g["c0"][d, sl]  # [8, 512]
            hh = g["h0"][d, sl]
            for j in range(4):
                c0l1[d, 32 * j:32 * j + BPC, :] = cc[:, 128 * j:128 * (j + 1)]
            h0l1[d] = hh.T
        m["c0_l1"] = c0l1
        m["h0T_l1"] = h0l1.astype(bf)
        c0l2 = np.zeros((128, 128), np.float32)
        h0l2 = np.zeros((2, H2, BPC), np.float32)
        for d in range(2):
            cc = g["c1"][d, sl]  # [8, 256]
            hh = g["h1"][d, sl]
            for j in range(2):
                q = 2 * d + j
                c0l2[32 * q:32 * q + BPC, :] = cc[:, 128 * j:128 * (j + 1)]
            h0l2[d] = hh.T
        m["c0_l2"] = c0l2
        m["h0T_l2"] = h0l2.astype(bf)
        in_maps.append(m)
    return in_maps


def kernel(**inputs) -> np.ndarray:
    import time
    from concourse.bass_utils import run_bass_kernel_spmd

    if "nc" not in _CACHE:
        _CACHE["nc"] = _build_program()
    nc = _CACHE["nc"]

    in_maps = _prep_inputs(inputs)
    res = None
    for attempt in range(3):
        try:
            res = run_bass_kernel_spmd(nc, in_maps, core_ids=list(range(8)))
            break
        except Exception:
            # transient NRT_EXEC_UNIT_UNRECOVERABLE after wedged runs —
            # observed to recover after ~60s
            if attempt == 2:
                raise
            time.sleep(60)
    loss = np.float64(0.0)
    for r in res.results:
        o = r["out"].astype(np.float64)  # [2, 8]: joint, logz
        loss += np.sum(o[0] - o[1])
    return np.float32(-loss)



# revision 7
# speedup vs baseline: 1.0363x; 1.0363x over previous
"""BiLSTM-CRF forward loss on 8 Trainium2 cores (batch-parallel SPMD).

Layout/sharding summary (per core, b=8 examples of B=64):
- embedding gather -> x^T (PE transposes) -> L1 input-projection GEMM (bf16)
- L1 BiLSTM scan: col-tiled state-stationary matmuls (4 strips of 8
  partitions), gates layout [128p=(4 strips x 8b), 512f=(i|f|o|g)*128]
- L2 BiLSTM scan: both directions packed in one 128-partition tile
- linear -> logits^T [48, T*8] and logits [T*8, 48]
- CRF forward pass in exp-domain: alpha_t = (expT^T @ alpha) * exp(emit_t),
  one bf16 matmul + one DVE mul per step; logZ via log of the final sum
- gold path score via indirect-DMA gathers + selector matmuls
Outputs per core: [2, 8] fp32 (row0 joint, row1 logZ). Host sums
-(joint - logZ) over all 64 examples.
"""

import numpy as np

B, T, VOCAB, EMBED, HID, TAGS = 64, 512, 30000, 512, 1024, 48
H1, H2 = HID // 2, HID // 4  # 512, 256
BPC = B // 8  # batch per core = 8
NTOK = T * BPC  # 4096 tokens per core
LN48 = float(np.log(48.0))

_CACHE = {}


def _gate_perm(h):
    """Permutation p such that W[p] has strip layout:
    strip j (512 cols) = [i_j | f_j | o_j | g_j], each 128 units of gate
    blocks taken from pytorch (i,f,g,o) row order. h = per-dir hidden."""
    nj = (4 * h) // 512
    slots = [0, 1, 3, 2]  # i, f, o, g source gate index
    p = []
    for j in range(nj):
        for g_idx in slots:
            base = g_idx * h + j * 128
            p.extend(range(base, base + 128))
    return np.array(p, dtype=np.int64)


def _build_program():
    import concourse.bass as bass
    import concourse.tile as tile
    import concourse.mybir as mybir
    from concourse.vector_clock import ScopedClock, VectorClock
    from concourse.masks import make_identity

    def _patched_drain_and_barrier(self, tick_clock, wait_clock):
        # This container's walrus rejects >2 sem waits on one CTRL
        # instruction; split the kernel-tail drain waits into per-proc
        # NOP waits on the same (in-order) SP queue.
        vc = tick_clock.global_clock
        n = len(vc)
        for p in range(n):
            t = vc[p]
            if t > 0:
                vec = [0] * n
                vec[p] = t
                nop = self.nc.sync.nop()
                wait_clock.add_sem_waits(nop.ins, ScopedClock({None: VectorClock(vec)}))
        self.nc.sync.drain()
        self.nc.all_engine_barrier()
        popped = self.nc._tile_sem_poison_stack.pop()
        assert popped is self._sem_poison
        self.nc.clear_and_free_semaphores(list(self.sems.allocated().values()))
        self.nc.all_engine_barrier()

    tile.TileContext._drain_and_barrier = _patched_drain_and_barrier

    f32 = mybir.dt.float32
    bf16 = mybir.dt.bfloat16
    i32 = mybir.dt.int32
    ACT = mybir.ActivationFunctionType
    ADD = mybir.AluOpType.add
    MULT = mybir.AluOpType.mult

    nc = bass.Bass()
    PH = int(__import__("os").environ.get("KPHASES", "99"))

    # ---------------- inputs ----------------
    def din(name, shape, dt=f32):
        return nc.dram_tensor(name, shape, dt, kind="ExternalInput")

    embed_bf = din("embed_bf", [VOCAB, EMBED], bf16)
    seq_tok = din("seq_tok", [NTOK, 1], i32)
    idx_emit = din("idx_emit", [NTOK, 1], i32)
    idx_trans = din("idx_trans", [NTOK, 1], i32)
    idx_start = din("idx_start", [BPC, 1], i32)
    idx_end = din("idx_end", [BPC, 1], i32)
    wihT1 = din("wihT1", [2, EMBED, 4 * H1], bf16)
    whhT1 = din("whhT1", [2, H1, 4 * H1], bf16)
    bias1 = din("bias1", [2, 1, 4 * H1], bf16)
    wihT2 = din("wihT2", [2, HID, 4 * H2], bf16)
    whhT2 = din("whhT2", [2, H2, 4 * H2], bf16)
    bias2 = din("bias2", [2, 1, 4 * H2], bf16)
    linWT = din("linWT", [H2 * 2, TAGS], bf16)
    lin_b = din("lin_b", [1, TAGS], bf16)
    c0_l1 = din("c0_l1", [2, 128, 128])  # strip-packed
    h0T_l1 = din("h0T_l1", [2, H1, BPC], bf16)
    c0_l2 = din("c0_l2", [2, 64, 128])  # per-dir strip-packed
    h0T_l2 = din("h0T_l2", [2, H2, BPC], bf16)
    trans_d = din("transitions", [TAGS, TAGS])
    start_d = din("start_trans", [TAGS, 1])
    end_d = din("end_trans", [TAGS, 1])
    sel_d = din("sel", [128, BPC])  # sel[p, b] = (p % 8 == b), fp32
    mask_d = din("mask_last", [128, 1])  # 1.0 except rows 120..127 -> 0.0

    out_d = nc.dram_tensor("out", [2, BPC], f32, kind="ExternalOutput")

    NM = NTOK // 128  # 32 token chunks

    with tile.TileContext(nc) as tc:
        with tc.tile_pool(name="dram", bufs=1, space="DRAM") as dpool, \
             tc.tile_pool(name="const", bufs=1) as cpool, \
             tc.tile_pool(name="persist", bufs=1) as ppool:

            xT_t = dpool.tile([EMBED, NTOK], bf16)          # 4 MB
            ih1_t = dpool.tile([2, T, 128, 512], bf16)      # 134 MB padded
            ih2_t = dpool.tile([T, 128, 512], bf16)         # 67 MB padded
            logits_t = dpool.tile([NTOK, TAGS], f32)

            ones_bf = cpool.tile([1, 512], bf16)
            nc.gpsimd.memset(ones_bf[:], 1.0)
            ones_f = cpool.tile([128, 1], f32)
            nc.gpsimd.memset(ones_f[:], 1.0)
            id8 = cpool.tile([128, 8], bf16)
            nc.gpsimd.memset(id8[:], 0.0)
            for j in range(4):
                make_identity(nc, id8[32 * j:32 * j + 8, :], nomemset=True)
            id128 = cpool.tile([128, 128], bf16)
            make_identity(nc, id128[:])
            sel_sb = cpool.tile([128, BPC], f32)
            nc.sync.dma_start(sel_sb[:], sel_d[:])
            mask_sb = cpool.tile([128, 1], f32)
            nc.sync.dma_start(mask_sb[:], mask_d[:])

            # h1T / h2T live in SBUF: [128, NTOK] bf16 tiles per 128-unit chunk
            h1T = [ppool.tile([128, NTOK], bf16, tag=f"h1T{i}", name=f"h1T{i}") for i in range(8)]
            h2T = [ppool.tile([128, NTOK], bf16, tag=f"h2T{i}", name=f"h2T{i}") for i in range(4)]

            # ================= P1: embedding gather + transpose =============
            with tc.tile_pool(name="p1", bufs=3) as sp, \
                 tc.tile_pool(name="p1p", bufs=4, space="PSUM") as psp:
                for m in range(NM if PH >= 1 else 0):
                    idx = sp.tile([128, 1], i32, tag="idx")
                    nc.sync.dma_start(idx[:], seq_tok[128 * m:128 * (m + 1), :])
                    xg = sp.tile([128, EMBED], bf16, tag="xg")
                    nc.gpsimd.indirect_dma_start(
                        out=xg[:], out_offset=None, in_=embed_bf[:],
                        in_offset=bass.IndirectOffsetOnAxis(ap=idx[:, :1], axis=0))
                    for e in range(EMBED // 128):
                        pt = psp.tile([128, 128], bf16, space="PSUM", tag="pt")
                        nc.tensor.transpose(out=pt[:], in_=xg[:, 128 * e:128 * (e + 1)],
                                            identity=id128[:])
                        xs = sp.tile([128, 128], bf16, tag="xs")
                        nc.vector.tensor_copy(xs[:], pt[:])
                        nc.sync.dma_start(
                            xT_t[128 * e:128 * (e + 1), 128 * m:128 * (m + 1)], xs[:])

            # ================= P2: L1 input GEMM ===========================
            with tc.tile_pool(name="w2", bufs=1) as wp, \
                 tc.tile_pool(name="p2", bufs=3) as sp, \
                 tc.tile_pool(name="p2p", bufs=4, space="PSUM") as psp:
                for d in range(2 if PH >= 2 else 0):
                    wt = [wp.tile([128, 4 * H1], bf16, tag=f"w1_{d}_{k}", name=f"w1_{d}_{k}") for k in range(4)]
                    for k in range(4):
                        nc.sync.dma_start(wt[k][:], wihT1[d, 128 * k:128 * (k + 1), :])
                    bt = wp.tile([1, 4 * H1], bf16, tag=f"b1_{d}")
                    nc.sync.dma_start(bt[:], bias1[d])
                    for m in range(NM):
                        xs = sp.tile([128, EMBED], bf16, tag="xs")
                        nc.sync.dma_start(
                            xs[:].rearrange("p (k n) -> p k n", k=4),
                            xT_t[:, 128 * m:128 * (m + 1)].rearrange(
                                "(k p) n -> p k n", p=128))
                        gsb = sp.tile([128, 4 * H1], bf16, tag="gsb")
                        for jj in range(4):
                            pg = psp.tile([128, 512], f32, space="PSUM", tag="pg")
                            for k in range(4):
                                nc.tensor.matmul(
                                    pg[:], lhsT=xs[:, 128 * k:128 * (k + 1)],
                                    rhs=wt[k][:, 512 * jj:512 * (jj + 1)],
                                    start=(k == 0), stop=False)
                            nc.tensor.matmul(
                                pg[:], lhsT=ones_bf[0:1, 0:128],
                                rhs=bt[0:1, 512 * jj:512 * (jj + 1)],
                                start=False, stop=True)
                            nc.scalar.copy(gsb[:, 512 * jj:512 * (jj + 1)], pg[:])
                        t0 = m * 16
                        for j in range(4):
                            dst = ih1_t[d, t0:t0 + 16].rearrange(
                                "t (j q) u -> t j q u", j=4)[:, j, 0:BPC, :]
                            nc.sync.dma_start(
                                dst, gsb[:, 512 * j:512 * (j + 1)])

            # ================= P3: L1 scans (fwd + bwd) ====================
            with tc.tile_pool(name="w3", bufs=1) as wp, \
                 tc.tile_pool(name="st3", bufs=1) as stp, \
                 tc.tile_pool(name="p3", bufs=4) as sp, \
                 tc.tile_pool(name="p3g", bufs=2, space="PSUM") as psg, \
                 tc.tile_pool(name="p3t", bufs=4, space="PSUM") as pst:
                whh = {}
                for d in range(2):
                    for k in range(4):
                        w = wp.tile([128, 4 * H1], bf16, tag=f"whh1_{d}_{k}")
                        nc.sync.dma_start(w[:], whhT1[d, 128 * k:128 * (k + 1), :])
                        whh[(d, k)] = w
                cS = {}
                hT0 = {}
                for d in range(2):
                    for par in range(2):
                        c = stp.tile([128, 128], f32, tag=f"c1_{d}_{par}")
                        cS[(d, par)] = c
                    nc.sync.dma_start(cS[(d, 0)][:], c0_l1[d])
                    h0 = stp.tile([128, 32], bf16, tag=f"h0T1_{d}")
                    for k in range(4):
                        nc.sync.dma_start(h0[:, 8 * k:8 * (k + 1)],
                                          h0T_l1[d, 128 * k:128 * (k + 1), :])
                    hT0[d] = h0

                for s in range(T if PH >= 3 else 0):
                    for d in range(2):
                        t = s if d == 0 else T - 1 - s
                        # lhsT source: previous hidden state transposed
                        if s == 0:
                            lhsT_of = lambda k, d=d: hT0[d][:, 8 * k:8 * (k + 1)]
                        else:
                            tp = (s - 1) if d == 0 else (T - s)
                            lhsT_of = (lambda k, d=d, tp=tp:
                                       h1T[d * 4 + k][:, 8 * tp:8 * tp + 8])
                        ih = sp.tile([128, 512], bf16, tag=f"ih1_{d}")
                        nc.sync.dma_start(ih[:], ih1_t[d, t])
                        pg = psg.tile([128, 512], f32, space="PSUM", tag=f"pg_{d}")
                        for jj in range(4):
                            for k in range(4):
                                nc.tensor.matmul(
                                    pg[32 * jj:32 * jj + 8, :], lhsT=lhsT_of(k),
                                    rhs=whh[(d, k)][:, 512 * jj:512 * (jj + 1)],
                                    start=(k == 0), stop=(k == 3),
                                    tile_position=(0, 32 * jj))
                        gsb = sp.tile([128, 512], bf16, tag=f"g1_{d}")
                        nc.vector.tensor_tensor(out=gsb[:], in0=pg[:], in1=ih[:], op=ADD)
                        sig = sp.tile([128, 384], bf16, tag=f"sig1_{d}")
                        nc.scalar.activation(sig[:], gsb[:, 0:384], ACT.Sigmoid)
                        tg = sp.tile([128, 128], bf16, tag=f"tg1_{d}")
                        nc.scalar.activation(tg[:], gsb[:, 384:512], ACT.Tanh)
                        c_old = cS[(d, s % 2)]
                        c_new = cS[(d, (s + 1) % 2)]
                        t1 = sp.tile([128, 128], f32, tag=f"t1_{d}")
                        nc.vector.tensor_tensor(out=t1[:], in0=sig[:, 128:256], in1=c_old[:], op=MULT)
                        t2 = sp.tile([128, 128], f32, tag=f"t2_{d}")
                        nc.vector.tensor_tensor(out=t2[:], in0=sig[:, 0:128], in1=tg[:], op=MULT)
                        nc.vector.tensor_tensor(out=c_new[:], in0=t1[:], in1=t2[:], op=ADD)
                        th = sp.tile([128, 128], bf16, tag=f"th1_{d}")
                        nc.scalar.activation(th[:], c_new[:], ACT.Tanh)
                        h = sp.tile([128, 128], bf16, tag=f"h1_{d}")
                        nc.vector.tensor_tensor(out=h[:], in0=sig[:, 256:384], in1=th[:], op=MULT)
                        for k in range(4):
                            pt = pst.tile([128, 8], bf16, space="PSUM", tag="pt3")
                            nc.tensor.transpose(out=pt[:], in_=h[32 * k:32 * k + 8, :],
                                                identity=id8[32 * k:32 * k + 8, :],
                                                tile_position=(32 * k, 0))
                            if k % 2 == 0:
                                nc.vector.tensor_copy(h1T[d * 4 + k][:, 8 * t:8 * t + 8], pt[:])
                            else:
                                nc.scalar.copy(h1T[d * 4 + k][:, 8 * t:8 * t + 8], pt[:])

            # ================= P4: L2 input GEMM ===========================
            with tc.tile_pool(name="w4", bufs=1) as wp, \
                 tc.tile_pool(name="p4", bufs=3) as sp, \
                 tc.tile_pool(name="p4p", bufs=4, space="PSUM") as psp:
                for d in range(2 if PH >= 4 else 0):
                    wt = [wp.tile([128, 4 * H2], bf16, tag=f"w2_{d}_{k}", name=f"w2_{d}_{k}") for k in range(8)]
                    for k in range(8):
                        nc.sync.dma_start(wt[k][:], wihT2[d, 128 * k:128 * (k + 1), :])
                    bt = wp.tile([1, 4 * H2], bf16, tag=f"b2_{d}")
                    nc.sync.dma_start(bt[:], bias2[d])
                    for m in range(NM):
                        gsb = sp.tile([128, 4 * H2], bf16, tag="g2sb")
                        for jj in range(2):
                            pg = psp.tile([128, 512], f32, space="PSUM", tag="pg4")
                            for k in range(8):
                                nc.tensor.matmul(
                                    pg[:], lhsT=h1T[k][:, 128 * m:128 * (m + 1)],
                                    rhs=wt[k][:, 512 * jj:512 * (jj + 1)],
                                    start=(k == 0), stop=False)
                            nc.tensor.matmul(
                                pg[:], lhsT=ones_bf[0:1, 0:128],
                                rhs=bt[0:1, 512 * jj:512 * (jj + 1)],
                                start=False, stop=True)
                            nc.scalar.copy(gsb[:, 512 * jj:512 * (jj + 1)], pg[:])
                        t0 = m * 16
                        for j in range(2):
                            dst = ih2_t[t0:t0 + 16].rearrange(
                                "t (j q) u -> t j q u", j=4)[:, 2 * d + j, 0:BPC, :]
                            nc.sync.dma_start(dst, gsb[:, 512 * j:512 * (j + 1)])

            # ================= P5: L2 scans (fwd + bwd independent) ========
            with tc.tile_pool(name="w5", bufs=1) as wp, \
                 tc.tile_pool(name="st5", bufs=1) as stp, \
                 tc.tile_pool(name="p5", bufs=4) as sp, \
                 tc.tile_pool(name="p5g", bufs=2, space="PSUM") as psg, \
                 tc.tile_pool(name="p5t", bufs=4, space="PSUM") as pst:
                whh2 = {}
                for d in range(2):
                    for k in range(2):
                        w = wp.tile([128, 4 * H2], bf16, tag=f"whh2_{d}_{k}")
                        nc.sync.dma_start(w[:], whhT2[d, 128 * k:128 * (k + 1), :])
                        whh2[(d, k)] = w
                c2 = {}
                h0_2 = {}
                for d in range(2):
                    for par in range(2):
                        c2[(d, par)] = stp.tile([64, 128], f32, tag=f"c2_{d}_{par}",
                                                name=f"c2_{d}_{par}")
                    nc.sync.dma_start(c2[(d, 0)][:], c0_l2[d])
                    h0 = stp.tile([128, 16], bf16, tag=f"h0T2_{d}")
                    for k in range(2):
                        nc.sync.dma_start(h0[:, 8 * k:8 * k + 8],
                                          h0T_l2[d, 128 * k:128 * (k + 1), :])
                    h0_2[d] = h0

                for s in range(T if PH >= 5 else 0):
                    for d in range(2):
                        t = s if d == 0 else T - 1 - s
                        ih = sp.tile([64, 512], bf16, tag=f"ih2_{d}")
                        nc.sync.dma_start(ih[:], ih2_t[t, 64 * d:64 * (d + 1), :])
                        pg = psg.tile([64, 512], f32, space="PSUM", tag=f"pg5_{d}")
                        for j in range(2):
                            for k in range(2):
                                if s == 0:
                                    lh = h0_2[d][:, 8 * k:8 * k + 8]
                                else:
                                    tp = (s - 1) if d == 0 else (T - s)
                                    lh = h2T[2 * d + k][:, 8 * tp:8 * tp + 8]
                                nc.tensor.matmul(
                                    pg[32 * j:32 * j + 8, :],
                                    lhsT=lh,
                                    rhs=whh2[(d, k)][:, 512 * j:512 * (j + 1)],
                                    start=(k == 0), stop=(k == 1),
                                    tile_position=(0, 32 * j))
                        gsb = sp.tile([64, 512], bf16, tag=f"g5_{d}")
                        nc.vector.tensor_tensor(out=gsb[:], in0=pg[:], in1=ih[:], op=ADD)
                        sig = sp.tile([64, 384], bf16, tag=f"sig5_{d}")
                        nc.scalar.activation(sig[:], gsb[:, 0:384], ACT.Sigmoid)
                        tg = sp.tile([64, 128], bf16, tag=f"tg5_{d}")
                        nc.scalar.activation(tg[:], gsb[:, 384:512], ACT.Tanh)
                        c_old, c_new = c2[(d, s % 2)], c2[(d, (s + 1) % 2)]
                        t1 = sp.tile([64, 128], f32, tag=f"t15_{d}")
                        nc.vector.tensor_tensor(out=t1[:], in0=sig[:, 128:256], in1=c_old[:], op=MULT)
                        t2 = sp.tile([64, 128], f32, tag=f"t25_{d}")
                        nc.vector.tensor_tensor(out=t2[:], in0=sig[:, 0:128], in1=tg[:], op=MULT)
                        nc.vector.tensor_tensor(out=c_new[:], in0=t1[:], in1=t2[:], op=ADD)
                        th = sp.tile([64, 128], bf16, tag=f"th5_{d}")
                        nc.scalar.activation(th[:], c_new[:], ACT.Tanh)
                        h = sp.tile([64, 128], bf16, tag=f"h5_{d}")
                        nc.vector.tensor_tensor(out=h[:], in0=sig[:, 256:384], in1=th[:], op=MULT)
                        for k in range(2):
                            q = 2 * d + k
                            pt = pst.tile([128, 8], bf16, space="PSUM", tag="pt5")
                            nc.tensor.transpose(out=pt[:], in_=h[32 * k:32 * k + 8, :],
                                                identity=id8[32 * k:32 * k + 8, :],
                                                tile_position=(32 * k, 0))
                            if q % 2 == 0:
                                nc.vector.tensor_copy(h2T[q][:, 8 * t:8 * t + 8], pt[:])
                            else:
                                nc.scalar.copy(h2T[q][:, 8 * t:8 * t + 8], pt[:])

            # ================= P6: linear -> logitsT + logits ==============
            logitsT = ppool.tile([TAGS, NTOK], f32, tag="logitsT")
            Esb = ppool.tile([TAGS, NTOK], bf16, tag="Esb")
            with tc.tile_pool(name="w6", bufs=1) as wp, \
                 tc.tile_pool(name="p6", bufs=3) as sp, \
                 tc.tile_pool(name="p6p", bufs=4, space="PSUM") as psp:
                lw = [wp.tile([128, TAGS], bf16, tag=f"lw{k}", name=f"lw{k}") for k in range(4)]
                for k in range(4):
                    nc.sync.dma_start(lw[k][:], linWT[128 * k:128 * (k + 1), :])
                lb = wp.tile([1, TAGS], bf16, tag="lb")
                nc.sync.dma_start(lb[:], lin_b[:])
                # logitsT [48, NTOK]
                for n in range(NTOK // 512 if PH >= 6 else 0):
                    pg = psp.tile([TAGS, 512], f32, space="PSUM", tag="pl")
                    for k in range(4):
                        nc.tensor.matmul(pg[:], lhsT=lw[k][:],
                                         rhs=h2T[k][:, 512 * n:512 * (n + 1)],
                                         start=(k == 0), stop=False)
                    nc.tensor.matmul(pg[:], lhsT=lb[0:1, :], rhs=ones_bf[0:1, :],
                                     start=False, stop=True)
                    nc.scalar.copy(logitsT[:, 512 * n:512 * (n + 1)], pg[:])
                    nc.scalar.activation(Esb[:, 512 * n:512 * (n + 1)],
                                         pg[:], ACT.Exp)
                # logits [NTOK, 48] to DRAM for gathers
                for m in range(NM if PH >= 6 else 0):
                    pg = psp.tile([128, TAGS], f32, space="PSUM", tag="pl2")
                    for k in range(4):
                        nc.tensor.matmul(pg[:], lhsT=h2T[k][:, 128 * m:128 * (m + 1)],
                                         rhs=lw[k][:], start=(k == 0), stop=False)
                    nc.tensor.matmul(pg[:], lhsT=ones_bf[0:1, 0:128], rhs=lb[0:1, :],
                                     start=False, stop=True)
                    ls = sp.tile([128, TAGS], f32, tag="ls")
                    nc.scalar.copy(ls[:], pg[:])
                    nc.sync.dma_start(logits_t[128 * m:128 * (m + 1), :], ls[:])

            # ================= P7: CRF partition (exp domain) ==============
            with tc.tile_pool(name="p7", bufs=1) as sp, \
                 tc.tile_pool(name="p7a", bufs=4) as ap, \
                 tc.tile_pool(name="p7p", bufs=2, space="PSUM") as psp, \
                 tc.tile_pool(name="p7f", bufs=1, space="PSUM") as psf:
                tr = sp.tile([TAGS, TAGS], f32, tag="tr")
                nc.sync.dma_start(tr[:], trans_d[:])
                ETp = sp.tile([TAGS, TAGS], bf16, tag="ETp")
                nln48 = sp.tile([TAGS, 1], f32, tag="nln48")
                nc.gpsimd.memset(nln48[:], -LN48)
                nc.scalar.activation(ETp[:], tr[:], ACT.Exp, bias=nln48[:, 0:1])
                stv = sp.tile([TAGS, 1], f32, tag="stv")
                nc.sync.dma_start(stv[:], start_d[:])
                env = sp.tile([TAGS, 1], f32, tag="env")
                nc.sync.dma_start(env[:], end_d[:])
                eend = sp.tile([TAGS, 1], bf16, tag="eend")
                nc.scalar.activation(eend[:], env[:], ACT.Exp)

                alpha = ap.tile([TAGS, BPC], bf16, tag="alpha")
                nc.scalar.activation(alpha[:], logitsT[:, 0:BPC], ACT.Exp,
                                     bias=stv[:, 0:1])
                for t in range(1, T if PH >= 7 else 1):
                    pm = psp.tile([TAGS, BPC], f32, space="PSUM", tag="pm")
                    nc.tensor.matmul(pm[:], lhsT=ETp[:], rhs=alpha[:],
                                     start=True, stop=True)
                    alpha = ap.tile([TAGS, BPC], bf16, tag="alpha")
                    nc.vector.tensor_tensor(out=alpha[:], in0=pm[:],
                                            in1=Esb[:, BPC * t:BPC * (t + 1)], op=MULT)
                pf = psf.tile([1, BPC], f32, space="PSUM", tag="pf")
                nc.tensor.matmul(pf[:], lhsT=eend[:], rhs=alpha[:],
                                 start=True, stop=True)
                logz = sp.tile([1, BPC], f32, tag="logz")
                nc.scalar.activation(logz[:], pf[:], ACT.Ln)
                nc.vector.tensor_scalar_add(logz[:], logz[:], float((T - 1) * LN48))
                nc.sync.dma_start(out_d[1:2, :], logz[:])

            # ================= P8: gold path score =========================
            with tc.tile_pool(name="p8", bufs=3) as sp, \
                 tc.tile_pool(name="p8a", bufs=1) as aw, \
                 tc.tile_pool(name="p8p", bufs=2, space="PSUM") as psp:
                accW = aw.tile([128, NM], f32, tag="accW")
                lfl = logits_t[:].rearrange("n k -> (n k)").unsqueeze(1)
                tfl = trans_d[:].rearrange("a b -> (a b)").unsqueeze(1)
                for m in range(NM if PH >= 8 else 0):
                    ie = sp.tile([128, 1], i32, tag="ie")
                    nc.sync.dma_start(ie[:], idx_emit[128 * m:128 * (m + 1), :])
                    it = sp.tile([128, 1], i32, tag="it")
                    nc.sync.dma_start(it[:], idx_trans[128 * m:128 * (m + 1), :])
                    ge = sp.tile([128, 1], f32, tag="ge")
                    nc.gpsimd.indirect_dma_start(
                        out=ge[:], out_offset=None, in_=lfl,
                        in_offset=bass.IndirectOffsetOnAxis(ap=ie[:, :1], axis=0))
                    gt = sp.tile([128, 1], f32, tag="gt")
                    nc.gpsimd.indirect_dma_start(
                        out=gt[:], out_offset=None, in_=tfl,
                        in_offset=bass.IndirectOffsetOnAxis(ap=it[:, :1], axis=0))
                    if m == NM - 1:
                        nc.vector.tensor_tensor(out=gt[:], in0=gt[:],
                                                in1=mask_sb[:], op=MULT)
                    nc.vector.tensor_tensor(out=accW[:, m:m + 1], in0=ge[:],
                                            in1=gt[:], op=ADD)
                se16 = sp.tile([16, 1], f32, tag="se16")
                ist = sp.tile([BPC, 1], i32, tag="ist")
                nc.sync.dma_start(ist[:], idx_start[:])
                nc.gpsimd.indirect_dma_start(
                    out=se16[0:8, :], out_offset=None,
                    in_=start_d[:].rearrange("a b -> (a b)").unsqueeze(1),
                    in_offset=bass.IndirectOffsetOnAxis(ap=ist[:, :1], axis=0))
                ien = sp.tile([BPC, 1], i32, tag="ien")
                nc.sync.dma_start(ien[:], idx_end[:])
                nc.gpsimd.indirect_dma_start(
                    out=se16[8:16, :], out_offset=None,
                    in_=end_d[:].rearrange("a b -> (a b)").unsqueeze(1),
                    in_offset=bass.IndirectOffsetOnAxis(ap=ien[:, :1], axis=0))
                s1 = psp.tile([NM, BPC], f32, space="PSUM", tag="s1")
                nc.tensor.matmul(s1[:], lhsT=accW[:], rhs=sel_sb[:],
                                 start=True, stop=True)
                s1s = sp.tile([NM, BPC], f32, tag="s1s")
                nc.scalar.copy(s1s[:], s1[:])
                s2 = psp.tile([1, BPC], f32, space="PSUM", tag="s2")
                nc.tensor.matmul(s2[:], lhsT=ones_f[0:NM, 0:1], rhs=s1s[:],
                                 start=True, stop=False)
                nc.tensor.matmul(s2[:], lhsT=se16[:], rhs=sel_sb[0:16, :],
                                 start=False, stop=True)
                joint = sp.tile([1, BPC], f32, tag="joint")
                nc.scalar.copy(joint[:], s2[:])
                nc.sync.dma_start(out_d[0:1, :], joint[:])

    _split_waits(nc, maxw=int(__import__("os").environ.get("KMAXW", "1")))
    return nc


def _split_waits(nc, maxw=2):
    """This container's walrus rejects instructions carrying more than a
    couple of semaphore waits. Hoist extras onto preceding same-engine
    NoOps (engines execute their stream in order, so this preserves the
    happens-before)."""
    import concourse.mybir as mybir
    import bass_rust
    compute_ops = {"Matmult", "Activation", "TensorTensor", "TensorScalar",
                   "TensorCopy", "TensorReduce", "Memset", "Iota",
                   "AffineSelect", "TensorTensorScan", "Select"}
    n_added = 0
    for fn in nc.m.functions:
        for blk in fn.blocks:
            insts = list(blk.instructions)
            out = []
            dirty = False
            for inst in insts:
                mw = 2 if (maxw == 0 and str(inst.opcode) in compute_ops) else max(1, maxw)
                si = inst.sync_info
                if si is not None and len(si.on_wait) > mw:
                    waits = list(si.on_wait)
                    extra, keep = waits[:-mw], waits[-mw:]
                    for i in range(0, len(extra), mw):
                        nop = mybir.InstNoOp(
                            name=f"{inst.name}_hw{i}", ins=[], outs=[])
                        nop.engine = inst.engine
                        nop.sync_info = bass_rust.SyncInfo(
                            on_wait=extra[i:i + mw], on_update=[])
                        out.append(nop)
                        n_added += 1
                    inst.sync_info = bass_rust.SyncInfo(
                        on_wait=keep, on_update=list(si.on_update))
                    dirty = True
                out.append(inst)
            if dirty:
                blk.instructions = out
    return n_added


def _prep_inputs(inputs):
    import ml_dtypes
    bf = ml_dtypes.bfloat16

    g = {k: np.asarray(v) for k, v in inputs.items()}
    seq = g["sequences"].astype(np.int64)
    tags = g["tags"].astype(np.int64)

    p1 = _gate_perm(H1)
    p2 = _gate_perm(H2)

    shared = {}
    shared["embed_bf"] = np.ascontiguousarray(g["embed_table"].astype(bf))
    for d, sfx in enumerate(["1f", "1b"]):
        wih = g["w_ih" + sfx][p1]  # [2048, 512] permuted rows
        whh = g["w_hh" + sfx][p1]
        b = (g["b_ih" + sfx] + g["b_hh" + sfx])[p1]
        shared.setdefault("wihT1", np.zeros((2, EMBED, 4 * H1), bf))[d] = wih.T.astype(bf)
        shared.setdefault("whhT1", np.zeros((2, H1, 4 * H1), bf))[d] = whh.T.astype(bf)
        shared.setdefault("bias1", np.zeros((2, 1, 4 * H1), bf))[d] = b.astype(bf)[None]
    for d, sfx in enumerate(["2f", "2b"]):
        wih = g["w_ih" + sfx][p2]
        whh = g["w_hh" + sfx][p2]
        b = (g["b_ih" + sfx] + g["b_hh" + sfx])[p2]
        shared.setdefault("wihT2", np.zeros((2, HID, 4 * H2), bf))[d] = wih.T.astype(bf)
        shared.setdefault("whhT2", np.zeros((2, H2, 4 * H2), bf))[d] = whh.T.astype(bf)
        shared.setdefault("bias2", np.zeros((2, 1, 4 * H2), bf))[d] = b.astype(bf)[None]
    shared["linWT"] = np.ascontiguousarray(g["lin_w"].T.astype(bf))
    shared["lin_b"] = g["lin_b"].astype(bf)[None, :]
    shared["transitions"] = g["transitions"].astype(np.float32)
    shared["start_trans"] = g["start_trans"].astype(np.float32)[:, None]
    shared["end_trans"] = g["end_trans"].astype(np.float32)[:, None]
    sel = (np.arange(128)[:, None] % 8 == np.arange(8)[None, :]).astype(np.float32)
    shared["sel"] = sel
    mask = np.ones((128, 1), np.float32)
    mask[120:128] = 0.0
    shared["mask_last"] = mask

    in_maps = []
    for c in range(8):
        b0 = c * BPC
        m = dict(shared)
        sl = slice(b0, b0 + BPC)
        seq_c = seq[sl]  # [8, T]
        tags_c = tags[sl]
        tok_seq = seq_c.T.reshape(NTOK).astype(np.int32)  # token-major (t, b)
        tok_tags = tags_c.T.reshape(NTOK).astype(np.int64)
        m["seq_tok"] = tok_seq[:, None]
        m["idx_emit"] = (np.arange(NTOK, dtype=np.int64) * TAGS + tok_tags).astype(np.int32)[:, None]
        nxt = np.concatenate([tok_tags[BPC:], np.zeros(BPC, np.int64)])
        it = tok_tags * TAGS + nxt
        it[-BPC:] = 0
        m["idx_trans"] = it.astype(np.int32)[:, None]
        m["idx_start"] = tags_c[:, 0].astype(np.int32)[:, None]
        m["idx_end"] = tags_c[:, T - 1].astype(np.int32)[:, None]

        c0l1 = np.zeros((2, 128, 128), np.float32)
        h0l1 = np.zeros((2, H1, BPC), np.float32)
        for d in range(2):
            cc = g["c0"][d, sl]  # [8, 512]
            hh = g["h0"][d, sl]
            for j in range(4):
                c0l1[d, 32 * j:32 * j + BPC, :] = cc[:, 128 * j:128 * (j + 1)]
            h0l1[d] = hh.T
        m["c0_l1"] = c0l1
        m["h0T_l1"] = h0l1.astype(bf)
        c0l2 = np.zeros((2, 64, 128), np.float32)
        h0l2 = np.zeros((2, H2, BPC), np.float32)
        for d in range(2):
            cc = g["c1"][d, sl]  # [8, 256]
            hh = g["h1"][d, sl]
            for j in range(2):
                c0l2[d, 32 * j:32 * j + BPC, :] = cc[:, 128 * j:128 * (j + 1)]
            h0l2[d] = hh.T
        m["c0_l2"] = c0l2
        m["h0T_l2"] = h0l2.astype(bf)
        in_maps.append(m)
    return in_maps


def kernel(**inputs) -> np.ndarray:
    import time
    from concourse.bass_utils import run_bass_kernel_spmd

    if "nc" not in _CACHE:
        _CACHE["nc"] = _build_program()
    nc = _CACHE["nc"]

    in_maps = _prep_inputs(inputs)
    res = None
    for attempt in range(3):
        try:
            res = run_bass_kernel_spmd(nc, in_maps, core_ids=list(range(8)))
            break
        except Exception:
            # transient NRT_EXEC_UNIT_UNRECOVERABLE after wedged runs —
            # observed to recover after ~60s
            if attempt == 2:
                raise
            time.sleep(60)
    loss = np.float64(0.0)
    for r in res.results:
        o = r["out"].astype(np.float64)  # [2, 8]: joint, logz
        loss += np.sum(o[0] - o[1])
    return np.float32(-loss)



# revision 14
# speedup vs baseline: 2.7091x; 2.6143x over previous
"""BiLSTM-CRF forward loss on 8 Trainium2 cores (batch-parallel SPMD).

v2 design — weight-stationary, transposed [units, batch] layout:
- embedding gather -> PE transpose -> xT8 [128, 4estrip, NTOK] fp8 (SBUF)
- input GEMMs and recurrent matmuls use fp8 DoubleRow (contraction 256/instr,
  0.5 cyc/row): stationary lhsT = weight tiles [128, 2, units],
  moving rhs = xT / h state [128, 2, tokens|batch]
- gates land in PSUM as [128 unit, (chunk, batch)]; per-step bias+ih come in
  via two identity matmuls (PSUM preload), so the serial chain is
  MM -> sigmoid -> (f*c, i*g) -> c_new -> tanh -> h  (no transposes, no adds)
- h written directly in transposed layout h1T[d] [128, 4j, T, 8b] fp8, which
  is both the next step's matmul operand and the next layer's GEMM input
- CRF partition in exp domain, 4 independent chains of 2 examples
  (alpha_t+1 = (ETp^T alpha) * exp(emit)), mult on DVE/Pool alternately
- gold path score computed on HOST from the returned logitsT (same logits the
  CRF used, so quantization errors cancel between joint and logZ)
Outputs per core: logitsT [48, NTOK] f32 and logz [1, 8] f32.
"""

import numpy as np

B, T, VOCAB, EMBED, HID, TAGS = 64, 512, 30000, 512, 1024, 48
H1, H2 = HID // 2, HID // 4  # 512, 256
BPC = B // 8  # 8 examples per core
NTOK = T * BPC  # 4096 tokens per core
LN48 = float(np.log(48.0))
GSLOT = [0, 1, 3, 2]  # our gate order (i,f,o,g) -> pytorch row block (i,f,g,o)

_CACHE = {}


def _gate_rows(h):
    """Row permutation: chunk c (=go*nj+jo) of 128 units covers pytorch rows
    pg*h + jo*128 + u, pg = GSLOT[go]. Chunks are gate-major so psum cols
    [i | f | o | g] with unit = 128*jo + p inside each gate block."""
    nj = h // 128
    out = np.empty(4 * h, np.int64)
    for c in range(4 * nj):
        go, jo = c // nj, c % nj
        pg = GSLOT[go]
        out[c * 128:(c + 1) * 128] = pg * h + jo * 128 + np.arange(128)
    return out


def _build_program():
    import concourse.bass as bass
    import concourse.tile as tile
    import concourse.mybir as mybir
    from concourse.vector_clock import ScopedClock, VectorClock
    from concourse.masks import make_identity

    def _patched_drain_and_barrier(self, tick_clock, wait_clock):
        # This container's walrus rejects >2 sem waits on one CTRL
        # instruction; split the kernel-tail drain waits into per-proc
        # NOP waits on the same (in-order) SP queue.
        vc = tick_clock.global_clock
        n = len(vc)
        for p in range(n):
            t = vc[p]
            if t > 0:
                vec = [0] * n
                vec[p] = t
                nop = self.nc.sync.nop()
                wait_clock.add_sem_waits(nop.ins, ScopedClock({None: VectorClock(vec)}))
        self.nc.sync.drain()
        self.nc.all_engine_barrier()
        popped = self.nc._tile_sem_poison_stack.pop()
        assert popped is self._sem_poison
        self.nc.clear_and_free_semaphores(list(self.sems.allocated().values()))
        self.nc.all_engine_barrier()

    tile.TileContext._drain_and_barrier = _patched_drain_and_barrier

    f32 = mybir.dt.float32
    bf16 = mybir.dt.bfloat16
    fp8 = mybir.dt.float8e4
    i32 = mybir.dt.int32
    ACT = mybir.ActivationFunctionType
    ADD = mybir.AluOpType.add
    MULT = mybir.AluOpType.mult
    DR = mybir.MatmulPerfMode.DoubleRow

    nc = bass.Bass()
    PH = int(__import__("os").environ.get("KPHASES", "99"))

    def din(name, shape, dt=f32):
        return nc.dram_tensor(name, shape, dt, kind="ExternalInput")

    embed_bf = din("embed_bf", [VOCAB, EMBED], bf16)
    seq_tok = din("seq_tok", [NTOK, 1], i32)
    wih1_d = din("wih1", [2, 2, 128, 2, 4 * H1], fp8)
    whh1_d = din("whh1", [2, 2, 128, 2, 4 * H1], fp8)
    biasg1_d = din("biasg1", [2, 128, 128], bf16)
    h01_d = din("h01", [2, 2, 128, 2, BPC], fp8)
    c01_d = din("c01", [2, 128, 32])
    wih2_d = din("wih2", [2, 4, 128, 2, 4 * H2], fp8)
    whh2_d = din("whh2", [2, 128, 2, 4 * H2], fp8)
    biasg2_d = din("biasg2", [2, 128, 64], bf16)
    h02_d = din("h02", [2, 128, 2, BPC], fp8)
    c02_d = din("c02", [2, 128, 16])
    linw_d = din("linw", [2, 128, 2, TAGS], fp8)
    linb_d = din("linb", [TAGS, 1])
    etp_d = din("etp", [TAGS, TAGS], bf16)
    start_d = din("start48", [TAGS, 1])
    ende_d = din("ende", [TAGS, 1], bf16)

    logitsT_d = nc.dram_tensor("logitsT", [TAGS, NTOK], f32, kind="ExternalOutput")
    logz_d = nc.dram_tensor("logz", [1, BPC], f32, kind="ExternalOutput")

    NM = NTOK // 128  # 32 gather chunks (16 timesteps each)

    with tile.TileContext(nc) as tc:
        with tc.tile_pool(name="dram", bufs=1, space="DRAM") as dpool, \
             tc.tile_pool(name="const", bufs=1) as cpool, \
             tc.tile_pool(name="persist", bufs=1) as ppool:

            # L1 pre-activations, (d, m) blocks of [128p, (16t, 16c, 8b)]
            ih1_t = dpool.tile([2, NM, 128, 2048], bf16)  # 33.5 MB
            # L2 pre-activations, (d, m) blocks of [128p, (64t, 8c, 8b)]
            ih2_t = dpool.tile([2, 8, 128, 4096], bf16)  # 16.8 MB

            id128 = cpool.tile([128, 128], bf16)
            make_identity(nc, id128[:])

            wih1sb, whh1sb, h01sb = {}, {}, {}
            for d in range(2):
                for kk in range(2):
                    w = cpool.tile([128, 2, 4 * H1], fp8, tag=f"wih1_{d}{kk}",
                                   name=f"wih1_{d}{kk}")
                    nc.sync.dma_start(w[:], wih1_d[d, kk])
                    wih1sb[(d, kk)] = w
                    w = cpool.tile([128, 2, 4 * H1], fp8, tag=f"whh1_{d}{kk}",
                                   name=f"whh1_{d}{kk}")
                    nc.sync.dma_start(w[:], whh1_d[d, kk])
                    whh1sb[(d, kk)] = w
                    h = cpool.tile([128, 2, BPC], fp8, tag=f"h01_{d}{kk}",
                                   name=f"h01_{d}{kk}")
                    nc.sync.dma_start(h[:], h01_d[d, kk])
                    h01sb[(d, kk)] = h
            wih2sb, biasg = {}, {}
            for d in range(2):
                for kk in range(4):
                    w = cpool.tile([128, 2, 4 * H2], fp8, tag=f"wih2_{d}{kk}",
                                   name=f"wih2_{d}{kk}")
                    nc.sync.dma_start(w[:], wih2_d[d, kk])
                    wih2sb[(d, kk)] = w
            whh2sb, h02sb = {}, {}
            for d in range(2):
                w = cpool.tile([128, 2, 4 * H2], fp8, tag=f"whh2_{d}", name=f"whh2_{d}")
                nc.sync.dma_start(w[:], whh2_d[d])
                whh2sb[d] = w
                h = cpool.tile([128, 2, BPC], fp8, tag=f"h02_{d}", name=f"h02_{d}")
                nc.sync.dma_start(h[:], h02_d[d])
                h02sb[d] = h
                bgl = cpool.tile([128, 128], bf16, tag=f"bg1_{d}", name=f"bg1_{d}")
                nc.sync.dma_start(bgl[:], biasg1_d[d])
                biasg[(1, d)] = bgl
                bgl = cpool.tile([128, 64], bf16, tag=f"bg2_{d}", name=f"bg2_{d}")
                nc.sync.dma_start(bgl[:], biasg2_d[d])
                biasg[(2, d)] = bgl
            linwsb = []
            for kk in range(2):
                w = cpool.tile([128, 2, TAGS], fp8, tag=f"linw_{kk}", name=f"linw_{kk}")
                nc.sync.dma_start(w[:], linw_d[kk])
                linwsb.append(w)
            linb_sb = cpool.tile([TAGS, 1], f32)
            nc.sync.dma_start(linb_sb[:], linb_d[:])
            etp_sb = cpool.tile([TAGS, TAGS], bf16)
            nc.sync.dma_start(etp_sb[:], etp_d[:])
            start_sb = cpool.tile([TAGS, 1], f32)
            nc.sync.dma_start(start_sb[:], start_d[:])
            ende_sb = cpool.tile([TAGS, 1], bf16)
            nc.sync.dma_start(ende_sb[:], ende_d[:])

            # persistent transposed activations
            h1T = [ppool.tile([128, 4, T, BPC], fp8, tag=f"h1T{d}", name=f"h1T{d}")
                   for d in range(2)]
            h2T = [ppool.tile([128, 2, T, BPC], fp8, tag=f"h2T{d}", name=f"h2T{d}")
                   for d in range(2)]

            # ====== P1+P2: embedding gather/transpose + L1 input GEMM ======
            with tc.tile_pool(name="px", bufs=1) as xpool, \
                 tc.tile_pool(name="p1", bufs=3) as sp, \
                 tc.tile_pool(name="p1s", bufs=3) as stp, \
                 tc.tile_pool(name="p1t", bufs=4, space="PSUM") as pst, \
                 tc.tile_pool(name="p1p", bufs=4, space="PSUM") as psp:
                xT8 = xpool.tile([128, 4, NTOK], fp8)
                for m in range(NM if PH >= 1 else 0):
                    idx = sp.tile([128, 1], i32, tag="idx")
                    nc.sync.dma_start(idx[:], seq_tok[128 * m:128 * (m + 1), :])
                    xg = sp.tile([128, EMBED], bf16, tag="xg")
                    nc.gpsimd.indirect_dma_start(
                        out=xg[:], out_offset=None, in_=embed_bf[:],
                        in_offset=bass.IndirectOffsetOnAxis(ap=idx[:, :1], axis=0))
                    for e in range(4):
                        pt = pst.tile([128, 128], bf16, space="PSUM", tag="pt")
                        nc.tensor.transpose(out=pt[:], in_=xg[:, 128 * e:128 * (e + 1)],
                                            identity=id128[:])
                        nc.vector.tensor_copy(xT8[:, e, 128 * m:128 * (m + 1)], pt[:])
                    if PH < 2:
                        continue
                    for d in range(2):
                        stg = stp.tile([128, 16, 16, BPC], bf16, tag=f"stg{d}")
                        for g in range(4):
                            pg4 = psp.tile([128, 4, 16, BPC], f32, space="PSUM",
                                           tag="pg2")
                            for cc in range(4):
                                c = 4 * g + cc
                                for kk in range(2):
                                    nc.tensor.matmul(
                                        pg4[:, cc, :, :],
                                        lhsT=wih1sb[(d, kk)][:, :, 128 * c:128 * (c + 1)],
                                        rhs=xT8[:, 2 * kk:2 * kk + 2,
                                                128 * m:128 * (m + 1)],
                                        start=(kk == 0), stop=(kk == 1),
                                        perf_mode=DR, skip_group_check=True)
                            src = pg4[:].rearrange("p c t b -> p t c b")
                            dst = stg[:, :, 4 * g:4 * (g + 1), :]
                            if g % 2 == 0:
                                nc.vector.tensor_copy(dst, src)
                            else:
                                nc.scalar.copy(dst, src)
                        nc.sync.dma_start(ih1_t[d, m], stg[:])

            # ================= P3: L1 scans (fwd + bwd) ====================
            with tc.tile_pool(name="st3", bufs=1) as stp, \
                 tc.tile_pool(name="ihp", bufs=3) as ihp, \
                 tc.tile_pool(name="p3", bufs=4) as sp, \
                 tc.tile_pool(name="p3g", bufs=2, space="PSUM") as psg:
                c1S = {}
                for d in range(2):
                    for par in range(2):
                        c1S[(d, par)] = stp.tile([128, 32], f32, tag=f"c1_{d}{par}",
                                                 name=f"c1_{d}{par}")
                    nc.sync.dma_start(c1S[(d, 0)][:], c01_d[d])

                ihm = {0: {}, 1: {}}

                def prefetch1(d, mb):
                    tl = ihp.tile([128, 16, 16, BPC], bf16, tag=f"ihm{d}",
                                  name=f"ihm{d}_{mb}")
                    nc.sync.dma_start(tl[:], ih1_t[d, mb])
                    ihm[d][mb] = tl

                if PH >= 3:
                    prefetch1(0, 0)
                    prefetch1(1, NM - 1)
                    prefetch1(0, 1)
                    prefetch1(1, NM - 2)
                for s in range(T if PH >= 3 else 0):
                    if s % 16 == 0 and s > 0:
                        mbf, mbb = s // 16 + 1, NM - 2 - s // 16
                        if mbf < NM:
                            prefetch1(0, mbf)
                        if mbb >= 0:
                            prefetch1(1, mbb)
                    for d in range(2):
                        t = s if d == 0 else T - 1 - s
                        mb, ti = t // 16, t % 16
                        pg = psg.tile([128, 128], f32, space="PSUM", tag=f"pg{d}",
                                      name=f"pg{d}_{s}")
                        nc.tensor.matmul(pg[:], lhsT=id128[:],
                                         rhs=ihm[d][mb][:, ti, :, :],
                                         start=True, stop=False, skip_group_check=True)
                        nc.tensor.matmul(pg[:], lhsT=id128[:], rhs=biasg[(1, d)][:],
                                         start=False, stop=False, skip_group_check=True)
                        for c in range(16):
                            for kk in range(2):
                                if s == 0:
                                    rh = h01sb[(d, kk)][:]
                                else:
                                    tp = (s - 1) if d == 0 else (T - s)
                                    rh = h1T[d][:, 2 * kk:2 * kk + 2, tp, :]
                                nc.tensor.matmul(
                                    pg[:, 8 * c:8 * (c + 1)],
                                    lhsT=whh1sb[(d, kk)][:, :, 128 * c:128 * (c + 1)],
                                    rhs=rh, start=False,
                                    stop=(c == 15 and kk == 1),
                                    perf_mode=DR, skip_group_check=True)
                        sig = sp.tile([128, 96], bf16, tag=f"sig{d}")
                        nc.scalar.activation(sig[:], pg[:, 0:96], ACT.Sigmoid)
                        tg = sp.tile([128, 32], bf16, tag=f"tg{d}")
                        nc.scalar.activation(tg[:], pg[:, 96:128], ACT.Tanh)
                        c_old, c_new = c1S[(d, s % 2)], c1S[(d, (s + 1) % 2)]
                        t1 = sp.tile([128, 32], f32, tag=f"t1_{d}")
                        nc.vector.tensor_tensor(out=t1[:], in0=sig[:, 32:64],
                                                in1=c_old[:], op=MULT)
                        t2 = sp.tile([128, 32], f32, tag=f"t2_{d}")
                        nc.gpsimd.tensor_tensor(out=t2[:], in0=sig[:, 0:32],
                                                in1=tg[:], op=MULT)
                        nc.vector.tensor_tensor(out=c_new[:], in0=t1[:], in1=t2[:],
                                                op=ADD)
                        th = sp.tile([128, 32], bf16, tag=f"th{d}")
                        nc.scalar.activation(th[:], c_new[:], ACT.Tanh)
                        nc.vector.tensor_tensor(
                            out=h1T[d][:, :, t, :],
                            in0=sig[:, 64:96].rearrange("p (j b) -> p j b", j=4),
                            in1=th[:].rearrange("p (j b) -> p j b", j=4), op=MULT)

            # ================= P4: L2 input GEMM ===========================
            with tc.tile_pool(name="p4s", bufs=3) as stp4, \
                 tc.tile_pool(name="p4p", bufs=4, space="PSUM") as psp:
                for d in range(2 if PH >= 4 else 0):
                    for m in range(8):
                        stg = stp4.tile([128, 64, 8, BPC], bf16, tag="stg4")
                        for c in range(8):
                            pg = psp.tile([128, 512], f32, space="PSUM", tag="pg4")
                            for kk in range(4):
                                rh = h1T[kk // 2][:, 2 * (kk % 2):2 * (kk % 2) + 2,
                                                  64 * m:64 * (m + 1), :]
                                nc.tensor.matmul(
                                    pg[:],
                                    lhsT=wih2sb[(d, kk)][:, :, 128 * c:128 * (c + 1)],
                                    rhs=rh, start=(kk == 0), stop=(kk == 3),
                                    perf_mode=DR, skip_group_check=True)
                            dst = stg[:, :, c, :]
                            src = pg[:].rearrange("p (t b) -> p t b", b=BPC)
                            if (c + m) % 2 == 0:
                                nc.vector.tensor_copy(dst, src)
                            else:
                                nc.scalar.copy(dst, src)
                        nc.sync.dma_start(ih2_t[d, m], stg[:])

            # ================= P5: L2 scans ================================
            with tc.tile_pool(name="st5", bufs=1) as stp, \
                 tc.tile_pool(name="ihp5", bufs=2) as ihp5, \
                 tc.tile_pool(name="p5", bufs=4) as sp, \
                 tc.tile_pool(name="p5g", bufs=2, space="PSUM") as psg:
                c2S = {}
                for d in range(2):
                    for par in range(2):
                        c2S[(d, par)] = stp.tile([128, 16], f32, tag=f"c2_{d}{par}",
                                                 name=f"c2_{d}{par}")
                    nc.sync.dma_start(c2S[(d, 0)][:], c02_d[d])

                ihm2 = {0: {}, 1: {}}

                def prefetch2(d, mb):
                    tl = ihp5.tile([128, 64, 8, BPC], bf16, tag=f"ihm2_{d}",
                                   name=f"ihm2_{d}_{mb}")
                    nc.sync.dma_start(tl[:], ih2_t[d, mb])
                    ihm2[d][mb] = tl

                if PH >= 5:
                    prefetch2(0, 0)
                    prefetch2(1, 7)
                for s in range(T if PH >= 5 else 0):
                    if s % 64 == 0 and s + 64 < T:
                        prefetch2(0, s // 64 + 1)
                        prefetch2(1, 6 - s // 64)
                    for d in range(2):
                        t = s if d == 0 else T - 1 - s
                        pg = psg.tile([128, 64], f32, space="PSUM", tag=f"pg5{d}",
                                      name=f"pg5{d}_{s}")
                        nc.tensor.matmul(pg[:], lhsT=id128[:],
                                         rhs=ihm2[d][t // 64][:, t % 64, :, :],
                                         start=True, stop=False, skip_group_check=True)
                        nc.tensor.matmul(pg[:], lhsT=id128[:], rhs=biasg[(2, d)][:],
                                         start=False, stop=False, skip_group_check=True)
                        for c in range(8):
                            if s == 0:
                                rh = h02sb[d][:]
                            else:
                                tp = (s - 1) if d == 0 else (T - s)
                                rh = h2T[d][:, :, tp, :]
                            nc.tensor.matmul(
                                pg[:, 8 * c:8 * (c + 1)],
                                lhsT=whh2sb[d][:, :, 128 * c:128 * (c + 1)],
                                rhs=rh, start=False, stop=(c == 7),
                                perf_mode=DR, skip_group_check=True)
                        sig = sp.tile([128, 48], bf16, tag=f"sg5{d}")
                        nc.scalar.activation(sig[:], pg[:, 0:48], ACT.Sigmoid)
                        tg = sp.tile([128, 16], bf16, tag=f"tg5{d}")
                        nc.scalar.activation(tg[:], pg[:, 48:64], ACT.Tanh)
                        c_old, c_new = c2S[(d, s % 2)], c2S[(d, (s + 1) % 2)]
                        t1 = sp.tile([128, 16], f32, tag=f"t15_{d}")
                        nc.vector.tensor_tensor(out=t1[:], in0=sig[:, 16:32],
                                                in1=c_old[:], op=MULT)
                        t2 = sp.tile([128, 16], f32, tag=f"t25_{d}")
                        nc.gpsimd.tensor_tensor(out=t2[:], in0=sig[:, 0:16],
                                                in1=tg[:], op=MULT)
                        nc.vector.tensor_tensor(out=c_new[:], in0=t1[:], in1=t2[:],
                                                op=ADD)
                        th = sp.tile([128, 16], bf16, tag=f"th5{d}")
                        nc.scalar.activation(th[:], c_new[:], ACT.Tanh)
                        nc.vector.tensor_tensor(
                            out=h2T[d][:, :, t, :],
                            in0=sig[:, 32:48].rearrange("p (j b) -> p j b", j=2),
                            in1=th[:].rearrange("p (j b) -> p j b", j=2), op=MULT)

            # ================= P6: linear -> logitsT + Esb =================
            logitsT_sb = ppool.tile([TAGS, NTOK], f32, tag="logitsT_sb")
            Esb = ppool.tile([TAGS, NTOK], bf16, tag="Esb")
            with tc.tile_pool(name="p6p", bufs=2, space="PSUM") as psp:
                for m in range(8 if PH >= 6 else 0):
                    pl = psp.tile([TAGS, 512], f32, space="PSUM", tag="pl")
                    for kk in range(2):
                        nc.tensor.matmul(pl[:], lhsT=linwsb[kk][:],
                                         rhs=h2T[kk][:, :, 64 * m:64 * (m + 1), :],
                                         start=(kk == 0), stop=(kk == 1),
                                         perf_mode=DR, skip_group_check=True)
                    nc.scalar.activation(logitsT_sb[:, 512 * m:512 * (m + 1)], pl[:],
                                         ACT.Identity, bias=linb_sb[:, 0:1])
                    nc.scalar.activation(Esb[:, 512 * m:512 * (m + 1)], pl[:],
                                         ACT.Exp, bias=linb_sb[:, 0:1])
                nc.sync.dma_start(logitsT_d[:], logitsT_sb[:])

            # ================= P7: CRF partition (exp domain) ==============
            NCH, CB = 4, BPC // 4
            with tc.tile_pool(name="p7", bufs=1) as sp, \
                 tc.tile_pool(name="p7a", bufs=3) as ap7, \
                 tc.tile_pool(name="p7p", bufs=1, space="PSUM") as psp:
                alpha = {}
                for ch in range(NCH if PH >= 7 else 0):
                    a0 = ap7.tile([TAGS, CB], bf16, tag=f"al{ch}", name=f"al{ch}_0")
                    nc.scalar.activation(a0[:], logitsT_sb[:, CB * ch:CB * (ch + 1)],
                                         ACT.Exp, bias=start_sb[:, 0:1])
                    alpha[ch] = a0
                for t in range(1, T if PH >= 7 else 1):
                    for ch in range(NCH):
                        pm = psp.tile([TAGS, CB], f32, space="PSUM", tag=f"pm{ch}",
                                      name=f"pm{ch}_{t}")
                        nc.tensor.matmul(pm[:], lhsT=etp_sb[:], rhs=alpha[ch][:],
                                         start=True, stop=True)
                        a = ap7.tile([TAGS, CB], bf16, tag=f"al{ch}",
                                     name=f"al{ch}_{t}")
                        eng = nc.vector if ch % 2 == 0 else nc.gpsimd
                        eng.tensor_tensor(
                            out=a[:], in0=pm[:],
                            in1=Esb[:, BPC * t + CB * ch:BPC * t + CB * (ch + 1)],
                            op=MULT)
                        alpha[ch] = a
                logz_sb = sp.tile([1, BPC], f32, tag="logz_sb")
                if PH >= 7:
                    for ch in range(NCH):
                        pf = psp.tile([1, CB], f32, space="PSUM", tag="pf",
                                      name=f"pf{ch}")
                        nc.tensor.matmul(pf[:], lhsT=ende_sb[:], rhs=alpha[ch][:],
                                         start=True, stop=True)
                        nc.scalar.activation(logz_sb[:, CB * ch:CB * (ch + 1)], pf[:],
                                             ACT.Ln)
                    nc.vector.tensor_scalar_add(logz_sb[:], logz_sb[:],
                                                float((T - 1) * LN48))
                else:
                    nc.gpsimd.memset(logz_sb[:], 0.0)
                nc.sync.dma_start(logz_d[:], logz_sb[:])

    _split_waits(nc, maxw=int(__import__("os").environ.get("KMAXW", "1")))
    return nc


def _split_waits(nc, maxw=2):
    """This container's walrus rejects instructions carrying more than a
    couple of semaphore waits. Hoist extras onto preceding same-engine
    NoOps (engines execute their stream in order, so this preserves the
    happens-before)."""
    import concourse.mybir as mybir
    import bass_rust
    compute_ops = {"Matmult", "Activation", "TensorTensor", "TensorScalar",
                   "TensorCopy", "TensorReduce", "Memset", "Iota",
                   "AffineSelect", "TensorTensorScan", "Select"}
    n_added = 0
    for fn in nc.m.functions:
        for blk in fn.blocks:
            insts = list(blk.instructions)
            out = []
            dirty = False
            for inst in insts:
                mw = 2 if (maxw == 0 and str(inst.opcode) in compute_ops) else max(1, maxw)
                si = inst.sync_info
                if si is not None and len(si.on_wait) > mw:
                    waits = list(si.on_wait)
                    extra, keep = waits[:-mw], waits[-mw:]
                    for i in range(0, len(extra), mw):
                        nop = mybir.InstNoOp(
                            name=f"{inst.name}_hw{i}", ins=[], outs=[])
                        nop.engine = inst.engine
                        nop.sync_info = bass_rust.SyncInfo(
                            on_wait=extra[i:i + mw], on_update=[])
                        out.append(nop)
                        n_added += 1
                    inst.sync_info = bass_rust.SyncInfo(
                        on_wait=keep, on_update=list(si.on_update))
                    dirty = True
                out.append(inst)
            if dirty:
                blk.instructions = out
    return n_added


def _prep_inputs(inputs):
    import ml_dtypes
    bf = ml_dtypes.bfloat16
    e4 = ml_dtypes.float8_e4m3fn

    g = {k: np.asarray(v) for k, v in inputs.items()}
    seq = g["sequences"].astype(np.int64)

    gr1 = _gate_rows(H1)  # 2048
    gr2 = _gate_rows(H2)  # 1024

    shared = {}
    shared["embed_bf"] = np.ascontiguousarray(g["embed_table"].astype(bf))
    wih1 = np.zeros((2, 2, 128, 2, 4 * H1), e4)
    whh1 = np.zeros((2, 2, 128, 2, 4 * H1), e4)
    biasg1 = np.zeros((2, 128, 128), bf)
    for d, sfx in enumerate(["1f", "1b"]):
        Wg = g["w_ih" + sfx][gr1].astype(np.float32)  # [2048, 512]
        wih1[d] = Wg.T.reshape(2, 2, 128, 4 * H1).transpose(0, 2, 1, 3).astype(e4)
        Wh = g["w_hh" + sfx][gr1].astype(np.float32)  # [2048, 512]
        whh1[d] = Wh.T.reshape(2, 2, 128, 4 * H1).transpose(0, 2, 1, 3).astype(e4)
        bb = (g["b_ih" + sfx] + g["b_hh" + sfx])[gr1].astype(np.float32)
        biasg1[d] = np.repeat(bb.reshape(16, 128).T[:, :, None], BPC,
                              axis=2).reshape(128, 128).astype(bf)
    wih2 = np.zeros((2, 4, 128, 2, 4 * H2), e4)
    whh2 = np.zeros((2, 128, 2, 4 * H2), e4)
    biasg2 = np.zeros((2, 128, 64), bf)
    for d, sfx in enumerate(["2f", "2b"]):
        Wg = g["w_ih" + sfx][gr2].astype(np.float32)  # [1024, 1024]
        wih2[d] = Wg.T.reshape(4, 2, 128, 4 * H2).transpose(0, 2, 1, 3).astype(e4)
        Wh = g["w_hh" + sfx][gr2].astype(np.float32)  # [1024, 256]
        whh2[d] = Wh.T.reshape(2, 128, 4 * H2).transpose(1, 0, 2).astype(e4)
        bb = (g["b_ih" + sfx] + g["b_hh" + sfx])[gr2].astype(np.float32)
        biasg2[d] = np.repeat(bb.reshape(8, 128).T[:, :, None], BPC,
                              axis=2).reshape(128, 64).astype(bf)
    shared["wih1"], shared["whh1"], shared["biasg1"] = wih1, whh1, biasg1
    shared["wih2"], shared["whh2"], shared["biasg2"] = wih2, whh2, biasg2
    lw = g["lin_w"].astype(np.float32)  # [48, 512]
    shared["linw"] = lw.T.reshape(2, 2, 128, TAGS).transpose(0, 2, 1, 3).astype(e4)
    shared["linb"] = g["lin_b"].astype(np.float32)[:, None]
    shared["etp"] = np.exp(g["transitions"].astype(np.float64) - LN48).astype(bf)
    shared["start48"] = g["start_trans"].astype(np.float32)[:, None]
    shared["ende"] = np.exp(g["end_trans"].astype(np.float64)).astype(bf)[:, None]

    in_maps = []
    for core in range(8):
        sl = slice(core * BPC, (core + 1) * BPC)
        m = dict(shared)
        m["seq_tok"] = seq[sl].T.reshape(NTOK).astype(np.int32)[:, None]
        h0 = g["h0"][:, sl].astype(np.float32)  # [2, 8, 512]
        m["h01"] = h0.reshape(2, BPC, 2, 2, 128).transpose(0, 2, 4, 3, 1).astype(e4)
        c0 = g["c0"][:, sl].astype(np.float32)
        m["c01"] = np.ascontiguousarray(
            c0.reshape(2, BPC, 4, 128).transpose(0, 3, 2, 1).reshape(2, 128, 32))
        h1 = g["h1"][:, sl].astype(np.float32)  # [2, 8, 256]
        m["h02"] = h1.reshape(2, BPC, 2, 128).transpose(0, 3, 2, 1).astype(e4)
        c1 = g["c1"][:, sl].astype(np.float32)
        m["c02"] = np.ascontiguousarray(
            c1.reshape(2, BPC, 2, 128).transpose(0, 3, 2, 1).reshape(2, 128, 16))
        in_maps.append(m)
    return in_maps


def kernel(**inputs) -> np.ndarray:
    import time
    from concourse.bass_utils import run_bass_kernel_spmd

    if "nc" not in _CACHE:
        _CACHE["nc"] = _build_program()
    nc = _CACHE["nc"]

    in_maps = _prep_inputs(inputs)
    res = None
    for attempt in range(3):
        try:
            res = run_bass_kernel_spmd(nc, in_maps, core_ids=list(range(8)))
            break
        except Exception:
            # transient NRT_EXEC_UNIT_UNRECOVERABLE after wedged runs —
            # observed to recover after ~60s
            if attempt == 2:
                raise
            time.sleep(60)

    tags = np.asarray(inputs["tags"]).astype(np.int64)
    mask = np.asarray(inputs["mask"]).astype(bool)
    trans = np.asarray(inputs["transitions"]).astype(np.float64)
    start = np.asarray(inputs["start_trans"]).astype(np.float64)
    end = np.asarray(inputs["end_trans"]).astype(np.float64)

    loss = np.float64(0.0)
    bidx = np.arange(BPC)
    for core, r in enumerate(res.results):
        lg = r["logitsT"].astype(np.float64)  # [48, NTOK]
        logz = r["logz"].astype(np.float64)[0]  # [8]
        sl = slice(core * BPC, (core + 1) * BPC)
        tt = tags[sl].T  # [T, 8]
        mt = mask[sl].T
        mf = mt.astype(np.float64)
        ltb = lg.reshape(TAGS, T, BPC).transpose(1, 2, 0)  # [T, 8, 48]
        emit = np.take_along_axis(ltb, tt[..., None], axis=-1)[..., 0]  # [T, 8]
        score = start[tt[0]]
        score = score + np.sum(trans[tt[:-1], tt[1:]] * mf[1:], axis=0)
        score = score + np.sum(emit[:-1] * mf[:-1], axis=0)
        last_idx = np.sum(mt.astype(np.int64), axis=0) - 1
        last_tags = tt[last_idx, bidx]
        score = score + end[last_tags] + ltb[-1, bidx, last_tags] * mf[-1]
        loss += np.sum(score - logz)
    return np.float32(-loss)


# revision 16
# speedup vs baseline: 2.7137x; 1.0017x over previous
"""BiLSTM-CRF forward loss on 8 Trainium2 cores (batch-parallel SPMD).

v2 design — weight-stationary, transposed [units, batch] layout:
- embedding gather -> PE transpose -> xT8 [128, 4estrip, NTOK] fp8 (SBUF)
- input GEMMs and recurrent matmuls use fp8 DoubleRow (contraction 256/instr,
  0.5 cyc/row): stationary lhsT = weight tiles [128, 2, units],
  moving rhs = xT / h state [128, 2, tokens|batch]
- gates land in PSUM as [128 unit, (chunk, batch)]; per-step bias+ih come in
  via two identity matmuls (PSUM preload), so the serial chain is
  MM -> sigmoid -> (f*c, i*g) -> c_new -> tanh -> h  (no transposes, no adds)
- h written directly in transposed layout h1T[d] [128, 4j, T, 8b] fp8, which
  is both the next step's matmul operand and the next layer's GEMM input
- CRF partition in exp domain, 4 independent chains of 2 examples
  (alpha_t+1 = (ETp^T alpha) * exp(emit)), mult on DVE/Pool alternately
- gold path score computed on HOST from the returned logitsT (same logits the
  CRF used, so quantization errors cancel between joint and logZ)
Outputs per core: logitsT [48, NTOK] f32 and logz [1, 8] f32.
"""

import numpy as np

B, T, VOCAB, EMBED, HID, TAGS = 64, 512, 30000, 512, 1024, 48
H1, H2 = HID // 2, HID // 4  # 512, 256
BPC = B // 8  # 8 examples per core
NTOK = T * BPC  # 4096 tokens per core
LN48 = float(np.log(48.0))
GSLOT = [0, 1, 3, 2]  # our gate order (i,f,o,g) -> pytorch row block (i,f,g,o)

_CACHE = {}


def _gate_rows(h):
    """Row permutation: chunk c (=go*nj+jo) of 128 units covers pytorch rows
    pg*h + jo*128 + u, pg = GSLOT[go]. Chunks are gate-major so psum cols
    [i | f | o | g] with unit = 128*jo + p inside each gate block."""
    nj = h // 128
    out = np.empty(4 * h, np.int64)
    for c in range(4 * nj):
        go, jo = c // nj, c % nj
        pg = GSLOT[go]
        out[c * 128:(c + 1) * 128] = pg * h + jo * 128 + np.arange(128)
    return out


def _build_program():
    import concourse.bass as bass
    import concourse.tile as tile
    import concourse.mybir as mybir
    from concourse.vector_clock import ScopedClock, VectorClock
    from concourse.masks import make_identity

    def _patched_drain_and_barrier(self, tick_clock, wait_clock):
        # This container's walrus rejects >2 sem waits on one CTRL
        # instruction; split the kernel-tail drain waits into per-proc
        # NOP waits on the same (in-order) SP queue.
        vc = tick_clock.global_clock
        n = len(vc)
        for p in range(n):
            t = vc[p]
            if t > 0:
                vec = [0] * n
                vec[p] = t
                nop = self.nc.sync.nop()
                wait_clock.add_sem_waits(nop.ins, ScopedClock({None: VectorClock(vec)}))
        self.nc.sync.drain()
        self.nc.all_engine_barrier()
        popped = self.nc._tile_sem_poison_stack.pop()
        assert popped is self._sem_poison
        self.nc.clear_and_free_semaphores(list(self.sems.allocated().values()))
        self.nc.all_engine_barrier()

    tile.TileContext._drain_and_barrier = _patched_drain_and_barrier

    f32 = mybir.dt.float32
    bf16 = mybir.dt.bfloat16
    fp8 = mybir.dt.float8e4
    i32 = mybir.dt.int32
    ACT = mybir.ActivationFunctionType
    ADD = mybir.AluOpType.add
    MULT = mybir.AluOpType.mult
    DR = mybir.MatmulPerfMode.DoubleRow

    nc = bass.Bass()
    PH = int(__import__("os").environ.get("KPHASES", "99"))

    def din(name, shape, dt=f32):
        return nc.dram_tensor(name, shape, dt, kind="ExternalInput")

    embed_bf = din("embed_bf", [VOCAB, EMBED], bf16)
    seq_tok = din("seq_tok", [NTOK, 1], i32)
    wih1_d = din("wih1", [2, 2, 128, 2, 4 * H1], fp8)
    whh1_d = din("whh1", [2, 2, 128, 2, 4 * H1], fp8)
    biasg1_d = din("biasg1", [2, 128, 128], bf16)
    h01_d = din("h01", [2, 2, 128, 2, BPC], fp8)
    c01_d = din("c01", [2, 128, 32])
    wih2_d = din("wih2", [2, 4, 128, 2, 4 * H2], fp8)
    whh2_d = din("whh2", [2, 128, 2, 4 * H2], fp8)
    biasg2_d = din("biasg2", [2, 128, 64], bf16)
    h02_d = din("h02", [2, 128, 2, BPC], fp8)
    c02_d = din("c02", [2, 128, 16])
    linw_d = din("linw", [2, 128, 2, TAGS], fp8)
    linb_d = din("linb", [TAGS, 1])
    etp_d = din("etp", [TAGS, TAGS], bf16)
    start_d = din("start48", [TAGS, 1])
    ende_d = din("ende", [TAGS, 1], bf16)

    logitsT_d = nc.dram_tensor("logitsT", [TAGS, NTOK], f32, kind="ExternalOutput")
    logz_d = nc.dram_tensor("logz", [1, BPC], f32, kind="ExternalOutput")

    NM = NTOK // 128  # 32 gather chunks (16 timesteps each)

    with tile.TileContext(nc) as tc:
        with tc.tile_pool(name="dram", bufs=1, space="DRAM") as dpool, \
             tc.tile_pool(name="const", bufs=1) as cpool, \
             tc.tile_pool(name="persist", bufs=1) as ppool:

            # L1 pre-activations, (d, m) blocks of [128p, (16t, 16c, 8b)]
            ih1_t = dpool.tile([2, NM, 128, 2048], bf16)  # 33.5 MB
            # L2 pre-activations, (d, m) blocks of [128p, (64t, 8c, 8b)]
            ih2_t = dpool.tile([2, 8, 128, 4096], bf16)  # 16.8 MB

            id128 = cpool.tile([128, 128], bf16)
            make_identity(nc, id128[:])

            wih1sb, whh1sb, h01sb = {}, {}, {}
            for d in range(2):
                for kk in range(2):
                    w = cpool.tile([128, 2, 4 * H1], fp8, tag=f"wih1_{d}{kk}",
                                   name=f"wih1_{d}{kk}")
                    nc.sync.dma_start(w[:], wih1_d[d, kk])
                    wih1sb[(d, kk)] = w
                    w = cpool.tile([128, 2, 4 * H1], fp8, tag=f"whh1_{d}{kk}",
                                   name=f"whh1_{d}{kk}")
                    nc.sync.dma_start(w[:], whh1_d[d, kk])
                    whh1sb[(d, kk)] = w
                    h = cpool.tile([128, 2, BPC], fp8, tag=f"h01_{d}{kk}",
                                   name=f"h01_{d}{kk}")
                    nc.sync.dma_start(h[:], h01_d[d, kk])
                    h01sb[(d, kk)] = h
            wih2sb, biasg = {}, {}
            for d in range(2):
                for kk in range(4):
                    w = cpool.tile([128, 2, 4 * H2], fp8, tag=f"wih2_{d}{kk}",
                                   name=f"wih2_{d}{kk}")
                    nc.sync.dma_start(w[:], wih2_d[d, kk])
                    wih2sb[(d, kk)] = w
            whh2sb, h02sb = {}, {}
            for d in range(2):
                w = cpool.tile([128, 2, 4 * H2], fp8, tag=f"whh2_{d}", name=f"whh2_{d}")
                nc.sync.dma_start(w[:], whh2_d[d])
                whh2sb[d] = w
                h = cpool.tile([128, 2, BPC], fp8, tag=f"h02_{d}", name=f"h02_{d}")
                nc.sync.dma_start(h[:], h02_d[d])
                h02sb[d] = h
                bgl = cpool.tile([128, 128], bf16, tag=f"bg1_{d}", name=f"bg1_{d}")
                nc.sync.dma_start(bgl[:], biasg1_d[d])
                biasg[(1, d)] = bgl
                bgl = cpool.tile([128, 64], bf16, tag=f"bg2_{d}", name=f"bg2_{d}")
                nc.sync.dma_start(bgl[:], biasg2_d[d])
                biasg[(2, d)] = bgl
            linwsb = []
            for kk in range(2):
                w = cpool.tile([128, 2, TAGS], fp8, tag=f"linw_{kk}", name=f"linw_{kk}")
                nc.sync.dma_start(w[:], linw_d[kk])
                linwsb.append(w)
            linb_sb = cpool.tile([TAGS, 1], f32)
            nc.sync.dma_start(linb_sb[:], linb_d[:])
            etp_sb = cpool.tile([TAGS, TAGS], bf16)
            nc.sync.dma_start(etp_sb[:], etp_d[:])
            start_sb = cpool.tile([TAGS, 1], f32)
            nc.sync.dma_start(start_sb[:], start_d[:])
            ende_sb = cpool.tile([TAGS, 1], bf16)
            nc.sync.dma_start(ende_sb[:], ende_d[:])

            # persistent transposed activations
            h1T = [ppool.tile([128, 4, T, BPC], fp8, tag=f"h1T{d}", name=f"h1T{d}")
                   for d in range(2)]
            h2T = [ppool.tile([128, 2, T, BPC], fp8, tag=f"h2T{d}", name=f"h2T{d}")
                   for d in range(2)]

            # ====== P1+P2: embedding gather/transpose + L1 input GEMM ======
            with tc.tile_pool(name="px", bufs=1) as xpool, \
                 tc.tile_pool(name="p1", bufs=3) as sp, \
                 tc.tile_pool(name="p1s", bufs=3) as stp, \
                 tc.tile_pool(name="p1t", bufs=4, space="PSUM") as pst, \
                 tc.tile_pool(name="p1p", bufs=4, space="PSUM") as psp:
                xT8 = xpool.tile([128, 4, NTOK], fp8)
                for m in range(NM if PH >= 1 else 0):
                    idx = sp.tile([128, 1], i32, tag="idx")
                    nc.sync.dma_start(idx[:], seq_tok[128 * m:128 * (m + 1), :])
                    xg = sp.tile([128, EMBED], bf16, tag="xg")
                    nc.gpsimd.indirect_dma_start(
                        out=xg[:], out_offset=None, in_=embed_bf[:],
                        in_offset=bass.IndirectOffsetOnAxis(ap=idx[:, :1], axis=0))
                    for e in range(4):
                        pt = pst.tile([128, 128], bf16, space="PSUM", tag="pt")
                        nc.tensor.transpose(out=pt[:], in_=xg[:, 128 * e:128 * (e + 1)],
                                            identity=id128[:])
                        nc.vector.tensor_copy(xT8[:, e, 128 * m:128 * (m + 1)], pt[:])
                    if PH < 2:
                        continue
                    for d in range(2):
                        stg = stp.tile([128, 16, 16, BPC], bf16, tag=f"stg{d}")
                        for g in range(4):
                            pg4 = psp.tile([128, 4, 16, BPC], f32, space="PSUM",
                                           tag="pg2")
                            for cc in range(4):
                                c = 4 * g + cc
                                for kk in range(2):
                                    nc.tensor.matmul(
                                        pg4[:, cc, :, :],
                                        lhsT=wih1sb[(d, kk)][:, :, 128 * c:128 * (c + 1)],
                                        rhs=xT8[:, 2 * kk:2 * kk + 2,
                                                128 * m:128 * (m + 1)],
                                        start=(kk == 0), stop=(kk == 1),
                                        perf_mode=DR, skip_group_check=True)
                            src = pg4[:].rearrange("p c t b -> p t c b")
                            dst = stg[:, :, 4 * g:4 * (g + 1), :]
                            if g % 2 == 0:
                                nc.vector.tensor_copy(dst, src)
                            else:
                                nc.scalar.copy(dst, src)
                        nc.sync.dma_start(ih1_t[d, m], stg[:])

            # ================= P3: L1 scans (fwd + bwd) ====================
            with tc.tile_pool(name="st3", bufs=1) as stp, \
                 tc.tile_pool(name="ihp", bufs=3) as ihp, \
                 tc.tile_pool(name="p3", bufs=4) as sp, \
                 tc.tile_pool(name="p3g", bufs=2, space="PSUM") as psg:
                c1S = {}
                for d in range(2):
                    for par in range(2):
                        c1S[(d, par)] = stp.tile([128, 32], f32, tag=f"c1_{d}{par}",
                                                 name=f"c1_{d}{par}")
                    nc.sync.dma_start(c1S[(d, 0)][:], c01_d[d])

                ihm = {0: {}, 1: {}}

                def prefetch1(d, mb):
                    tl = ihp.tile([128, 16, 16, BPC], bf16, tag=f"ihm{d}",
                                  name=f"ihm{d}_{mb}")
                    nc.sync.dma_start(tl[:], ih1_t[d, mb])
                    ihm[d][mb] = tl

                if PH >= 3:
                    prefetch1(0, 0)
                    prefetch1(1, NM - 1)
                    prefetch1(0, 1)
                    prefetch1(1, NM - 2)
                for s in range(T if PH >= 3 else 0):
                    if s % 16 == 0 and s > 0:
                        mbf, mbb = s // 16 + 1, NM - 2 - s // 16
                        if mbf < NM:
                            prefetch1(0, mbf)
                        if mbb >= 0:
                            prefetch1(1, mbb)
                    # stage-interleaved across the two direction chains so the
                    # in-order engine queues advance both in lockstep
                    pg, sig, tg, t1, t2, th = {}, {}, {}, {}, {}, {}
                    for d in range(2):
                        t = s if d == 0 else T - 1 - s
                        mb, ti = t // 16, t % 16
                        pg[d] = psg.tile([128, 128], f32, space="PSUM", tag=f"pg{d}",
                                         name=f"pg{d}_{s}")
                        nc.tensor.matmul(pg[d][:], lhsT=id128[:],
                                         rhs=ihm[d][mb][:, ti, :, :],
                                         start=True, stop=False, skip_group_check=True)
                        nc.tensor.matmul(pg[d][:], lhsT=id128[:], rhs=biasg[(1, d)][:],
                                         start=False, stop=False, skip_group_check=True)
                        for c in range(16):
                            for kk in range(2):
                                if s == 0:
                                    rh = h01sb[(d, kk)][:]
                                else:
                                    tp = (s - 1) if d == 0 else (T - s)
                                    rh = h1T[d][:, 2 * kk:2 * kk + 2, tp, :]
                                nc.tensor.matmul(
                                    pg[d][:, 8 * c:8 * (c + 1)],
                                    lhsT=whh1sb[(d, kk)][:, :, 128 * c:128 * (c + 1)],
                                    rhs=rh, start=False,
                                    stop=(c == 15 and kk == 1),
                                    perf_mode=DR, skip_group_check=True)
                    for d in range(2):
                        sig[d] = sp.tile([128, 96], bf16, tag=f"sig{d}",
                                         name=f"sig{d}_{s}")
                        nc.scalar.activation(sig[d][:], pg[d][:, 0:96], ACT.Sigmoid)
                    for d in range(2):
                        tg[d] = sp.tile([128, 32], bf16, tag=f"tg{d}",
                                        name=f"tg{d}_{s}")
                        nc.scalar.activation(tg[d][:], pg[d][:, 96:128], ACT.Tanh)
                    for d in range(2):
                        t1[d] = sp.tile([128, 32], f32, tag=f"t1_{d}",
                                        name=f"t1_{d}_{s}")
                        nc.vector.tensor_tensor(out=t1[d][:], in0=sig[d][:, 32:64],
                                                in1=c1S[(d, s % 2)][:], op=MULT)
                        t2[d] = sp.tile([128, 32], f32, tag=f"t2_{d}",
                                        name=f"t2_{d}_{s}")
                        nc.gpsimd.tensor_tensor(out=t2[d][:], in0=sig[d][:, 0:32],
                                                in1=tg[d][:], op=MULT)
                    for d in range(2):
                        nc.vector.tensor_tensor(out=c1S[(d, (s + 1) % 2)][:],
                                                in0=t1[d][:], in1=t2[d][:], op=ADD)
                    for d in range(2):
                        th[d] = sp.tile([128, 32], bf16, tag=f"th{d}",
                                        name=f"th{d}_{s}")
                        nc.scalar.activation(th[d][:], c1S[(d, (s + 1) % 2)][:],
                                             ACT.Tanh)
                    for d in range(2):
                        t = s if d == 0 else T - 1 - s
                        nc.vector.tensor_tensor(
                            out=h1T[d][:, :, t, :],
                            in0=sig[d][:, 64:96].rearrange("p (j b) -> p j b", j=4),
                            in1=th[d][:].rearrange("p (j b) -> p j b", j=4), op=MULT)

            # ================= P4: L2 input GEMM ===========================
            with tc.tile_pool(name="p4s", bufs=3) as stp4, \
                 tc.tile_pool(name="p4p", bufs=4, space="PSUM") as psp:
                for d in range(2 if PH >= 4 else 0):
                    for m in range(8):
                        stg = stp4.tile([128, 64, 8, BPC], bf16, tag="stg4")
                        for c in range(8):
                            pg = psp.tile([128, 512], f32, space="PSUM", tag="pg4")
                            for kk in range(4):
                                rh = h1T[kk // 2][:, 2 * (kk % 2):2 * (kk % 2) + 2,
                                                  64 * m:64 * (m + 1), :]
                                nc.tensor.matmul(
                                    pg[:],
                                    lhsT=wih2sb[(d, kk)][:, :, 128 * c:128 * (c + 1)],
                                    rhs=rh, start=(kk == 0), stop=(kk == 3),
                                    perf_mode=DR, skip_group_check=True)
                            dst = stg[:, :, c, :]
                            src = pg[:].rearrange("p (t b) -> p t b", b=BPC)
                            if (c + m) % 2 == 0:
                                nc.vector.tensor_copy(dst, src)
                            else:
                                nc.scalar.copy(dst, src)
                        nc.sync.dma_start(ih2_t[d, m], stg[:])

            # ================= P5: L2 scans ================================
            with tc.tile_pool(name="st5", bufs=1) as stp, \
                 tc.tile_pool(name="ihp5", bufs=2) as ihp5, \
                 tc.tile_pool(name="p5", bufs=4) as sp, \
                 tc.tile_pool(name="p5g", bufs=2, space="PSUM") as psg:
                c2S = {}
                for d in range(2):
                    for par in range(2):
                        c2S[(d, par)] = stp.tile([128, 16], f32, tag=f"c2_{d}{par}",
                                                 name=f"c2_{d}{par}")
                    nc.sync.dma_start(c2S[(d, 0)][:], c02_d[d])

                ihm2 = {0: {}, 1: {}}

                def prefetch2(d, mb):
                    tl = ihp5.tile([128, 64, 8, BPC], bf16, tag=f"ihm2_{d}",
                                   name=f"ihm2_{d}_{mb}")
                    nc.sync.dma_start(tl[:], ih2_t[d, mb])
                    ihm2[d][mb] = tl

                if PH >= 5:
                    prefetch2(0, 0)
                    prefetch2(1, 7)
                for s in range(T if PH >= 5 else 0):
                    if s % 64 == 0 and s + 64 < T:
                        prefetch2(0, s // 64 + 1)
                        prefetch2(1, 6 - s // 64)
                    for d in range(2):
                        t = s if d == 0 else T - 1 - s
                        pg = psg.tile([128, 64], f32, space="PSUM", tag=f"pg5{d}",
                                      name=f"pg5{d}_{s}")
                        nc.tensor.matmul(pg[:], lhsT=id128[:],
                                         rhs=ihm2[d][t // 64][:, t % 64, :, :],
                                         start=True, stop=False, skip_group_check=True)
                        nc.tensor.matmul(pg[:], lhsT=id128[:], rhs=biasg[(2, d)][:],
                                         start=False, stop=False, skip_group_check=True)
                        for c in range(8):
                            if s == 0:
                                rh = h02sb[d][:]
                            else:
                                tp = (s - 1) if d == 0 else (T - s)
                                rh = h2T[d][:, :, tp, :]
                            nc.tensor.matmul(
                                pg[:, 8 * c:8 * (c + 1)],
                                lhsT=whh2sb[d][:, :, 128 * c:128 * (c + 1)],
                                rhs=rh, start=False, stop=(c == 7),
                                perf_mode=DR, skip_group_check=True)
                        sig = sp.tile([128, 48], bf16, tag=f"sg5{d}")
                        nc.scalar.activation(sig[:], pg[:, 0:48], ACT.Sigmoid)
                        tg = sp.tile([128, 16], bf16, tag=f"tg5{d}")
                        nc.scalar.activation(tg[:], pg[:, 48:64], ACT.Tanh)
                        c_old, c_new = c2S[(d, s % 2)], c2S[(d, (s + 1) % 2)]
                        t1 = sp.tile([128, 16], f32, tag=f"t15_{d}")
                        nc.vector.tensor_tensor(out=t1[:], in0=sig[:, 16:32],
                                                in1=c_old[:], op=MULT)
                        t2 = sp.tile([128, 16], f32, tag=f"t25_{d}")
                        nc.gpsimd.tensor_tensor(out=t2[:], in0=sig[:, 0:16],
                                                in1=tg[:], op=MULT)
                        nc.vector.tensor_tensor(out=c_new[:], in0=t1[:], in1=t2[:],
                                                op=ADD)
                        th = sp.tile([128, 16], bf16, tag=f"th5{d}")
                        nc.scalar.activation(th[:], c_new[:], ACT.Tanh)
                        nc.vector.tensor_tensor(
                            out=h2T[d][:, :, t, :],
                            in0=sig[:, 32:48].rearrange("p (j b) -> p j b", j=2),
                            in1=th[:].rearrange("p (j b) -> p j b", j=2), op=MULT)

            # ================= P6: linear -> logitsT + Esb =================
            logitsT_sb = ppool.tile([TAGS, NTOK], f32, tag="logitsT_sb")
            Esb = ppool.tile([TAGS, NTOK], bf16, tag="Esb")
            with tc.tile_pool(name="p6p", bufs=2, space="PSUM") as psp:
                for m in range(8 if PH >= 6 else 0):
                    pl = psp.tile([TAGS, 512], f32, space="PSUM", tag="pl")
                    for kk in range(2):
                        nc.tensor.matmul(pl[:], lhsT=linwsb[kk][:],
                                         rhs=h2T[kk][:, :, 64 * m:64 * (m + 1), :],
                                         start=(kk == 0), stop=(kk == 1),
                                         perf_mode=DR, skip_group_check=True)
                    nc.scalar.activation(logitsT_sb[:, 512 * m:512 * (m + 1)], pl[:],
                                         ACT.Identity, bias=linb_sb[:, 0:1])
                    nc.scalar.activation(Esb[:, 512 * m:512 * (m + 1)], pl[:],
                                         ACT.Exp, bias=linb_sb[:, 0:1])
                nc.sync.dma_start(logitsT_d[:], logitsT_sb[:])

            # ================= P7: CRF partition (exp domain) ==============
            NCH, CB = 4, BPC // 4
            with tc.tile_pool(name="p7", bufs=1) as sp, \
                 tc.tile_pool(name="p7a", bufs=3) as ap7, \
                 tc.tile_pool(name="p7p", bufs=1, space="PSUM") as psp:
                alpha = {}
                for ch in range(NCH if PH >= 7 else 0):
                    a0 = ap7.tile([TAGS, CB], bf16, tag=f"al{ch}", name=f"al{ch}_0")
                    nc.scalar.activation(a0[:], logitsT_sb[:, CB * ch:CB * (ch + 1)],
                                         ACT.Exp, bias=start_sb[:, 0:1])
                    alpha[ch] = a0
                for t in range(1, T if PH >= 7 else 1):
                    for ch in range(NCH):
                        pm = psp.tile([TAGS, CB], f32, space="PSUM", tag=f"pm{ch}",
                                      name=f"pm{ch}_{t}")
                        nc.tensor.matmul(pm[:], lhsT=etp_sb[:], rhs=alpha[ch][:],
                                         start=True, stop=True)
                        a = ap7.tile([TAGS, CB], bf16, tag=f"al{ch}",
                                     name=f"al{ch}_{t}")
                        eng = nc.vector
                        eng.tensor_tensor(
                            out=a[:], in0=pm[:],
                            in1=Esb[:, BPC * t + CB * ch:BPC * t + CB * (ch + 1)],
                            op=MULT)
                        alpha[ch] = a
                logz_sb = sp.tile([1, BPC], f32, tag="logz_sb")
                if PH >= 7:
                    for ch in range(NCH):
                        pf = psp.tile([1, CB], f32, space="PSUM", tag="pf",
                                      name=f"pf{ch}")
                        nc.tensor.matmul(pf[:], lhsT=ende_sb[:], rhs=alpha[ch][:],
                                         start=True, stop=True)
                        nc.scalar.activation(logz_sb[:, CB * ch:CB * (ch + 1)], pf[:],
                                             ACT.Ln)
                    nc.vector.tensor_scalar_add(logz_sb[:], logz_sb[:],
                                                float((T - 1) * LN48))
                else:
                    nc.gpsimd.memset(logz_sb[:], 0.0)
                nc.sync.dma_start(logz_d[:], logz_sb[:])

    _split_waits(nc, maxw=int(__import__("os").environ.get("KMAXW", "1")))
    return nc


def _split_waits(nc, maxw=2):
    """This container's walrus rejects instructions carrying more than a
    couple of semaphore waits. Hoist extras onto preceding same-engine
    NoOps (engines execute their stream in order, so this preserves the
    happens-before)."""
    import concourse.mybir as mybir
    import bass_rust
    compute_ops = {"Matmult", "Activation", "TensorTensor", "TensorScalar",
                   "TensorCopy", "TensorReduce", "Memset", "Iota",
                   "AffineSelect", "TensorTensorScan", "Select"}
    n_added = 0
    for fn in nc.m.functions:
        for blk in fn.blocks:
            insts = list(blk.instructions)
            out = []
            dirty = False
            for inst in insts:
                mw = 2 if (maxw == 0 and str(inst.opcode) in compute_ops) else max(1, maxw)
                si = inst.sync_info
                if si is not None and len(si.on_wait) > mw:
                    waits = list(si.on_wait)
                    extra, keep = waits[:-mw], waits[-mw:]
                    for i in range(0, len(extra), mw):
                        nop = mybir.InstNoOp(
                            name=f"{inst.name}_hw{i}", ins=[], outs=[])
                        nop.engine = inst.engine
                        nop.sync_info = bass_rust.SyncInfo(
                            on_wait=extra[i:i + mw], on_update=[])
                        out.append(nop)
                        n_added += 1
                    inst.sync_info = bass_rust.SyncInfo(
                        on_wait=keep, on_update=list(si.on_update))
                    dirty = True
                out.append(inst)
            if dirty:
                blk.instructions = out
    return n_added


def _prep_inputs(inputs):
    import ml_dtypes
    bf = ml_dtypes.bfloat16
    e4 = ml_dtypes.float8_e4m3fn

    g = {k: np.asarray(v) for k, v in inputs.items()}
    seq = g["sequences"].astype(np.int64)

    gr1 = _gate_rows(H1)  # 2048
    gr2 = _gate_rows(H2)  # 1024

    shared = {}
    shared["embed_bf"] = np.ascontiguousarray(g["embed_table"].astype(bf))
    wih1 = np.zeros((2, 2, 128, 2, 4 * H1), e4)
    whh1 = np.zeros((2, 2, 128, 2, 4 * H1), e4)
    biasg1 = np.zeros((2, 128, 128), bf)
    for d, sfx in enumerate(["1f", "1b"]):
        Wg = g["w_ih" + sfx][gr1].astype(np.float32)  # [2048, 512]
        wih1[d] = Wg.T.reshape(2, 2, 128, 4 * H1).transpose(0, 2, 1, 3).astype(e4)
        Wh = g["w_hh" + sfx][gr1].astype(np.float32)  # [2048, 512]
        whh1[d] = Wh.T.reshape(2, 2, 128, 4 * H1).transpose(0, 2, 1, 3).astype(e4)
        bb = (g["b_ih" + sfx] + g["b_hh" + sfx])[gr1].astype(np.float32)
        biasg1[d] = np.repeat(bb.reshape(16, 128).T[:, :, None], BPC,
                              axis=2).reshape(128, 128).astype(bf)
    wih2 = np.zeros((2, 4, 128, 2, 4 * H2), e4)
    whh2 = np.zeros((2, 128, 2, 4 * H2), e4)
    biasg2 = np.zeros((2, 128, 64), bf)
    for d, sfx in enumerate(["2f", "2b"]):
        Wg = g["w_ih" + sfx][gr2].astype(np.float32)  # [1024, 1024]
        wih2[d] = Wg.T.reshape(4, 2, 128, 4 * H2).transpose(0, 2, 1, 3).astype(e4)
        Wh = g["w_hh" + sfx][gr2].astype(np.float32)  # [1024, 256]
        whh2[d] = Wh.T.reshape(2, 128, 4 * H2).transpose(1, 0, 2).astype(e4)
        bb = (g["b_ih" + sfx] + g["b_hh" + sfx])[gr2].astype(np.float32)
        biasg2[d] = np.repeat(bb.reshape(8, 128).T[:, :, None], BPC,
                              axis=2).reshape(128, 64).astype(bf)
    shared["wih1"], shared["whh1"], shared["biasg1"] = wih1, whh1, biasg1
    shared["wih2"], shared["whh2"], shared["biasg2"] = wih2, whh2, biasg2
    lw = g["lin_w"].astype(np.float32)  # [48, 512]
    shared["linw"] = lw.T.reshape(2, 2, 128, TAGS).transpose(0, 2, 1, 3).astype(e4)
    shared["linb"] = g["lin_b"].astype(np.float32)[:, None]
    shared["etp"] = np.exp(g["transitions"].astype(np.float64) - LN48).astype(bf)
    shared["start48"] = g["start_trans"].astype(np.float32)[:, None]
    shared["ende"] = np.exp(g["end_trans"].astype(np.float64)).astype(bf)[:, None]

    in_maps = []
    for core in range(8):
        sl = slice(core * BPC, (core + 1) * BPC)
        m = dict(shared)
        m["seq_tok"] = seq[sl].T.reshape(NTOK).astype(np.int32)[:, None]
        h0 = g["h0"][:, sl].astype(np.float32)  # [2, 8, 512]
        m["h01"] = h0.reshape(2, BPC, 2, 2, 128).transpose(0, 2, 4, 3, 1).astype(e4)
        c0 = g["c0"][:, sl].astype(np.float32)
        m["c01"] = np.ascontiguousarray(
            c0.reshape(2, BPC, 4, 128).transpose(0, 3, 2, 1).reshape(2, 128, 32))
        h1 = g["h1"][:, sl].astype(np.float32)  # [2, 8, 256]
        m["h02"] = h1.reshape(2, BPC, 2, 128).transpose(0, 3, 2, 1).astype(e4)
        c1 = g["c1"][:, sl].astype(np.float32)
        m["c02"] = np.ascontiguousarray(
            c1.reshape(2, BPC, 2, 128).transpose(0, 3, 2, 1).reshape(2, 128, 16))
        in_maps.append(m)
    return in_maps


def kernel(**inputs) -> np.ndarray:
    import time
    from concourse.bass_utils import run_bass_kernel_spmd

    if "nc" not in _CACHE:
        _CACHE["nc"] = _build_program()
    nc = _CACHE["nc"]

    in_maps = _prep_inputs(inputs)
    res = None
    for attempt in range(3):
        try:
            res = run_bass_kernel_spmd(nc, in_maps, core_ids=list(range(8)))
            break
        except Exception:
            # transient NRT_EXEC_UNIT_UNRECOVERABLE after wedged runs —
            # observed to recover after ~60s
            if attempt == 2:
                raise
            time.sleep(60)

    tags = np.asarray(inputs["tags"]).astype(np.int64)
    mask = np.asarray(inputs["mask"]).astype(bool)
    trans = np.asarray(inputs["transitions"]).astype(np.float64)
    start = np.asarray(inputs["start_trans"]).astype(np.float64)
    end = np.asarray(inputs["end_trans"]).astype(np.float64)

    loss = np.float64(0.0)
    bidx = np.arange(BPC)
    for core, r in enumerate(res.results):
        lg = r["logitsT"].astype(np.float64)  # [48, NTOK]
        logz = r["logz"].astype(np.float64)[0]  # [8]
        sl = slice(core * BPC, (core + 1) * BPC)
        tt = tags[sl].T  # [T, 8]
        mt = mask[sl].T
        mf = mt.astype(np.float64)
        ltb = lg.reshape(TAGS, T, BPC).transpose(1, 2, 0)  # [T, 8, 48]
        emit = np.take_along_axis(ltb, tt[..., None], axis=-1)[..., 0]  # [T, 8]
        score = start[tt[0]]
        score = score + np.sum(trans[tt[:-1], tt[1:]] * mf[1:], axis=0)
        score = score + np.sum(emit[:-1] * mf[:-1], axis=0)
        last_idx = np.sum(mt.astype(np.int64), axis=0) - 1
        last_tags = tt[last_idx, bidx]
        score = score + end[last_tags] + ltb[-1, bidx, last_tags] * mf[-1]
        loss += np.sum(score - logz)
    return np.float32(-loss)
